# revision 1
# baseline (speedup 1.0000x reference)
"""DenseCL loss kernel for 8 TRN2 NeuronCores.

Sharding: core c owns batch image c for the dense branch + correspondence,
queue rows [c*8192, (c+1)*8192) for the queue-InfoNCE negatives, and rows
[c*784, (c+1)*784) of the flat dense-InfoNCE logits.  Collectives: AllGather
of pooled features (global head inputs), AllGather of matched keys (logits
columns), final AllReduce of partial scalars.
"""
import sys

if "/opt/trn_rl_repo" not in sys.path:
    sys.path.insert(0, "/opt/trn_rl_repo")

import numpy as np
import ml_dtypes

import concourse.bass as bass
import concourse.bacc as bacc
import concourse.mybir as mybir
import concourse.tile as tile
from concourse.tile import add_dep_helper
from concourse import bass_utils, masks

BF = ml_dtypes.bfloat16
F32 = mybir.dt.float32
BF16 = mybir.dt.bfloat16
F8 = mybir.dt.float8e4

N_CORES = 8
B, HW, C, D, P, Q = 8, 784, 1024, 2048, 128, 65536
QSH = Q // N_CORES          # 8192 queue rows per core
CT, DT = C // 128, D // 128  # 8, 16
NT = B * HW                 # 6272 total dense rows
TAU = 0.2
LAM = 0.5
ISC = 1.0 / TAU             # 5.0
AF = mybir.ActivationFunctionType
ALU = mybir.AluOpType

# 784 = 6*128 + 16 partition tiles
PT = [(i * 128, min(128, HW - i * 128)) for i in range(7)]


def _chunks(n, step=512):
    return [(o, min(step, n - o)) for o in range(0, n, step)]


def _patch_act_tables():
    """Force every activation we use onto the natural_log_exp_and_others
    table set so the kernel needs exactly one ACT_TABLE_LOAD."""
    from concourse import hw_specs
    import concourse.bacc as bacc_mod
    if getattr(bacc_mod, "_act_tables_patched", False):
        return
    orig = hw_specs.get_activation_tables
    ours = {AF.Exp, AF.Ln, AF.Relu, AF.Identity, AF.Copy, AF.Square}
    keep = "natural_log_exp_and_others"

    def patched(arch):
        tabs = orig(arch)
        assert keep in tabs and ours <= tabs[keep]
        return {name: (fns if name == keep else fns - ours)
                for name, fns in tabs.items()}

    bacc_mod.get_activation_tables = patched
    bacc_mod._act_tables_patched = True


def _build(do_compile=True):
    _patch_act_tables()
    nc = bacc.Bacc("TRN2", target_bir_lowering=False, debug=False,
                   num_devices=N_CORES)

    def inp(name, shape, dt):
        return nc.dram_tensor(name, list(shape), dt, kind="ExternalInput")

    xq_d = inp("xq", (128, CT * HW), BF16)       # [c, ct*784+p] = feat_q[b, p, ct*128+c]
    xk_d = inp("xk", (128, CT * HW), BF16)
    wd1_d = inp("wd1", (DT, 128, C), BF16)       # [dt, c, ct*128+d]
    wd1m_d = inp("wd1m", (DT, 128, C), BF16)
    wd2_d = inp("wd2", (128, D), BF16)           # [c, dt*128+d] = Wd2[dt*128+c, d]
    wd2m_d = inp("wd2m", (128, D), BF16)
    wg1_d = inp("wg1", (128, CT * D), BF16)      # [c, ct*2048+d] = Wg1[ct*128+c, d]
    wg1m_d = inp("wg1m", (128, CT * D), BF16)
    wg2_d = inp("wg2", (128, D), BF16)           # like wd2
    wg2m_d = inp("wg2m", (128, D), BF16)
    bd1_d = inp("bd1", (128, DT), F32)           # [r, dt] = bd1[dt*128+r]
    bd1m_d = inp("bd1m", (128, DT), F32)
    bd2_d = inp("bd2", (128, 1), F32)
    bd2m_d = inp("bd2m", (128, 1), F32)
    bg1_d = inp("bg1", (128, DT), F32)
    bg1m_d = inp("bg1m", (128, DT), F32)
    bg2_d = inp("bg2", (128, 1), F32)
    bg2m_d = inp("bg2m", (128, 1), F32)
    queueT_d = inp("queueT", (128, QSH), BF16)   # [ch, j] = queue[c0+j, ch]
    iota_d = inp("iota", (128, 1), F32)          # 0..127
    onesc_d = inp("onesc", (128, 1), F32)        # ones column (lhsT for partition sums)
    onesr_d = inp("onesr", (1, 128), F32)        # ones row (lhsT for K=1 broadcast)
    ones8_d = inp("ones8", (1, 8), BF16)         # lhsT for bias broadcast (K=1, M=8)

    out_d = nc.dram_tensor("out", [1, 32], F32, kind="ExternalOutput")

    with tile.TileContext(nc) as tc:
        rg = [list(range(N_CORES))]
        # ---------------- DRAM bounce buffers for collectives ----------------
        with tc.tile_pool(name="dramp", bufs=1, space="DRAM") as dpool:
            pool_in = dpool.tile([2 * C], F32, name="pool_in")
            pool_out = dpool.tile([N_CORES * 2 * C], F32, name="pool_out",
                                  addr_space="Shared")
            match_in = dpool.tile([128 * HW], F8, name="match_in")
            match_mid = dpool.tile([2 * 128 * HW], F8, name="match_mid")
            match_out = dpool.tile([N_CORES * 128 * HW], F8, name="match_out")
            fin_in = dpool.tile([32], F32, name="fin_in")
            fin_out = dpool.tile([32], F32, name="fin_out", addr_space="Shared")

            _main(nc, tc, rg, locals())
    if do_compile:
        nc.compile()
    return nc


def _main(nc, tc, rg, env):
    g = lambda k: env[k]
    xq_d, xk_d = g("xq_d"), g("xk_d")
    pool_in, pool_out = g("pool_in"), g("pool_out")
    match_in, match_out = g("match_in"), g("match_out")
    fin_in, fin_out = g("fin_in"), g("fin_out")

    # ---------------- constants / inputs into SBUF ----------------
    with tc.tile_pool(name="cst", bufs=1) as cst:
        _body(nc, tc, rg, env, cst)


def _body(nc, tc, rg, env, cst):
    g = lambda k: env[k]
    xq_d, xk_d = g("xq_d"), g("xk_d")
    pool_in, pool_out = g("pool_in"), g("pool_out")
    match_in, match_out = g("match_in"), g("match_out")
    match_mid = g("match_mid")
    fin_in, fin_out = g("fin_in"), g("fin_out")

    def load(name, shape, dt, src=None, eng=None):
        t = cst.tile(list(shape), dt, name=name + "_sb")
        (eng or nc.sync).dma_start(t[:], (src if src is not None else
                                          g(name + "_d"))[:])
        return t

    # critical-path inputs first: per-ct X tiles so matmuls can start as
    # soon as the first 200 KB lands (sync ring feeds the q branch, scalar
    # ring feeds the k branch / later weights)
    xq_t, xk_t = [], []
    w1t_pre = None
    for ct in range(CT):
        t = cst.tile([128, HW], BF16, name=f"xq{ct}_sb")
        nc.sync.dma_start(t[:], xq_d[:, ct * HW:(ct + 1) * HW])
        xq_t.append(t)
        if ct == 0:
            w1t_pre = cst.tile([128, C], BF16, name="w1t_pre")
            nc.sync.dma_start(w1t_pre[:], g("wd1_d")[0, :, :])
    wd2_sb = load("wd2", (128, D), BF16, eng=nc.scalar)
    for ct in range(CT):
        xk_t.append(cst.tile([128, HW], BF16, name=f"xk{ct}_sb"))
    wd2m_sb = cst.tile([128, D], BF16, name="wd2m_sb")
    # small constants on the gpsimd (SWDGE) ring so they don't delay X/W
    iota_sb = load("iota", (128, 1), F32, eng=nc.gpsimd)
    onesc_sb = load("onesc", (128, 1), F32, eng=nc.gpsimd)
    onesr_sb = load("onesr", (1, 128), F32, eng=nc.gpsimd)
    ones8_sb = load("ones8", (1, 8), BF16, eng=nc.gpsimd)
    bd1_sb = load("bd1", (128, DT), F32, eng=nc.gpsimd)
    bd1m_sb = load("bd1m", (128, DT), F32, eng=nc.gpsimd)
    bd2_sb = load("bd2", (128, 1), F32, eng=nc.gpsimd)
    bd2m_sb = load("bd2m", (128, 1), F32, eng=nc.gpsimd)
    bg1_sb = load("bg1", (128, DT), F32, eng=nc.gpsimd)
    bg1m_sb = load("bg1m", (128, DT), F32, eng=nc.gpsimd)
    bg2_sb = load("bg2", (128, 1), F32, eng=nc.gpsimd)
    bg2m_sb = load("bg2m", (128, 1), F32, eng=nc.gpsimd)
    onescb_sb = cst.tile([128, 1], BF16, name="onescb_sb")
    nc.vector.tensor_copy(onescb_sb[:], onesc_sb[:])
    id_f = cst.tile([128, 128], F32, name="id_f")
    masks.make_identity(nc, id_f[:])
    id_b = cst.tile([128, 128], BF16, name="id_b")
    masks.make_identity(nc, id_b[:])

    # long-lived results
    qdT_bf = cst.tile([128, HW], BF16, name="qdT_bf")    # normalized q_d.T (own image)
    kdT_bf = cst.tile([128, HW], BF16, name="kdT_bf")
    qgT_bf = cst.tile([128, 8], BF16, name="qgT_bf")     # normalized q_g.T (all 8 images)
    kgT_bf = cst.tile([128, 8], BF16, name="kgT_bf")
    lpos_sb = cst.tile([1, 8], F32, name="lpos_sb")      # q_g . k_g per image
    qsums_sb = cst.tile([1, 8], F32, name="qsums_sb")    # partial sum(exp(l_neg/tau))
    matchT_sb = cst.tile([128, HW], BF16, name="matchT_sb")
    matchT_f8 = cst.tile([128, HW], F8, name="matchT_f8")
    qdT_f8 = cst.tile([128, HW], F8, name="qdT_f8")

    queueT_sb = cst.tile([128, QSH], BF16, name="queueT_sb")

    # =============== dense heads (bulk matmuls) ===============
    with tc.tile_pool(name="w1p", bufs=8) as w1p, \
         tc.tile_pool(name="hp", bufs=3) as hp, \
         tc.tile_pool(name="l2s", bufs=2) as l2s, \
         tc.tile_pool(name="ps_big", bufs=2, space="PSUM") as ps_big, \
         tc.tile_pool(name="ps_qd", bufs=2, space="PSUM") as ps_qd:
        for br, (xs, w1n, w2sb, b1, b2, dst) in enumerate([
                (xq_t, "wd1", wd2_sb, bd1_sb, bd2_sb, qdT_bf),
                (xk_t, "wd1m", wd2m_sb, bd1m_sb, bd2m_sb, kdT_bf)]):
            w1_d = g(w1n + "_d")
            qd_ps = ps_qd.tile([128, HW], F32, name="qd_ps", tag="qd")
            for dt in range(DT):
                if br == 0 and dt == 0:
                    w1t = w1t_pre
                else:
                    w1t = w1p.tile([128, C], BF16, name="w1t")
                    nc.sync.dma_start(w1t[:], w1_d[dt, :, :])
                h_ps = ps_big.tile([128, HW], F32, name="h_ps", tag="big")
                for ct in range(CT):
                    for (o, n) in _chunks(HW):
                        nc.tensor.matmul(
                            h_ps[:, o:o + n],
                            lhsT=w1t[:, ct * 128:(ct + 1) * 128],
                            rhs=xs[ct][:, o:o + n],
                            start=(ct == 0), stop=(ct == CT - 1))
                h_sb = hp.tile([128, HW], BF16, name="h_sb")
                relu_bi = nc.scalar.activation(h_sb[:], h_ps[:], AF.Relu,
                                               bias=b1[:, dt:dt + 1])
                if br == 0 and dt in (2, 5, 8, 11):
                    # emit the k-branch input DMAs here: the scalar engine
                    # stream reaches them only after the q pipeline's first
                    # relus, so they don't steal HBM bandwidth from xq/wd1
                    c0 = 2 * ((dt - 2) // 3)
                    for ct2 in range(c0, c0 + 2):
                        nc.scalar.dma_start(
                            xk_t[ct2][:],
                            xk_d[:, ct2 * HW:(ct2 + 1) * HW])
                if br == 0 and dt == 13:
                    nc.scalar.dma_start(wd2m_sb[:], g("wd2m_d")[:])
                    # pooling + its AllGather, emitted here so the
                    # DVE ops fill the idle window during the dense heads
                    with tc.tile_pool(name="plp", bufs=2) as plp:
                        pool_sb = cst.tile([128, 16], F32, name="pool_sb")
                        for src, base in ((xq_t, 0), (xk_t, 8)):
                            for ct2 in range(CT):
                                scr = plp.tile([128, HW], BF16,
                                               name="pool_scr")
                                nc.vector.tensor_scalar(
                                    scr[:], src[ct2][:], 1.0, None,
                                    op0=ALU.mult, op1=ALU.add,
                                    accum_out=pool_sb[:, base + ct2:
                                                      base + ct2 + 1])
                        pin = pool_in[:].rearrange("(g t c) -> c (g t)",
                                                   g=2, t=8, c=128)
                        nc.gpsimd.dma_start(pin, pool_sb[:])
                    nc.gpsimd.collective_compute(
                        "AllGather", ALU.bypass, replica_groups=rg,
                        ins=[pool_in.opt()], outs=[pool_out.opt()])
                for (o, n) in _chunks(HW):
                    nc.tensor.matmul(
                        qd_ps[:, o:o + n],
                        lhsT=w2sb[:, dt * 128:(dt + 1) * 128],
                        rhs=h_sb[:, o:o + n],
                        start=(dt == 0), stop=(dt == DT - 1))
            # bias + l2-normalize along channels (partition dim)
            qdT_f = l2s.tile([128, HW], F32, name="qdT_f")
            nc.scalar.activation(qdT_f[:], qd_ps[:], AF.Identity, bias=b2[:])
            sq = l2s.tile([128, HW], BF16, name="sq")
            nc.scalar.activation(sq[:], qdT_f[:], AF.Square)
            ssq_ps = ps_qd.tile([1, HW], F32, name="ssq_ps", tag="qd")
            for (o, n) in _chunks(HW):
                nc.tensor.matmul(ssq_ps[:, o:o + n], lhsT=onescb_sb[:],
                                 rhs=sq[:, o:o + n], start=True, stop=True)
            nrm = l2s.tile([1, HW], F32, name="nrm")
            nc.vector.tensor_scalar_max(nrm[:], ssq_ps[:], 1e-12)
            # rsqrt(s) = exp(-0.5*ln(s)) — keeps ACT on one table set
            nrm2 = l2s.tile([1, HW], F32, name="nrm2")
            nc.scalar.activation(nrm2[:], nrm[:], AF.Ln)
            rn = l2s.tile([1, HW], F32, name="rn")
            nc.scalar.activation(rn[:], nrm2[:], AF.Exp, scale=-0.5)
            rnb_ps = ps_qd.tile([128, HW], F32, name="rnb_ps", tag="qd")
            for (o, n) in _chunks(HW):
                nc.tensor.matmul(rnb_ps[:, o:o + n], lhsT=onesr_sb[:],
                                 rhs=rn[:, o:o + n], start=True, stop=True)
            nc.vector.tensor_mul(dst[:], qdT_f[:], rnb_ps[:])

    # =============== dense correspondence (own image) ===============
    with tc.tile_pool(name="cor", bufs=1) as cor:
        sim_sb = cor.tile([128, 7 * HW], F32, name="sim_sb")
        with tc.tile_pool(name="ps_sim", bufs=2, space="PSUM") as ps_sim:
            for i, (po, pn) in enumerate(PT):
                s_ps = ps_sim.tile([128, HW], F32, name="s_ps")
                for (o, n) in _chunks(HW):
                    nc.tensor.matmul(s_ps[0:pn, o:o + n],
                                     lhsT=qdT_bf[:, po:po + pn],
                                     rhs=kdT_bf[:, o:o + n],
                                     start=True, stop=True)
                nc.scalar.activation(sim_sb[0:pn, i * HW:i * HW + HW],
                                     s_ps[0:pn, :], AF.Copy)
        mx8 = cor.tile([128, 8], F32, name="mx8")
        ix8 = cor.tile([128, 8], mybir.dt.uint32, name="ix8")
        ixf = cor.tile([128, 7], F32, name="ixf")
        for i, (po, pn) in enumerate(PT):
            nc.vector.max(mx8[0:pn, :], sim_sb[0:pn, i * HW:i * HW + HW])
            nc.vector.max_index(ix8[0:pn, :], mx8[0:pn, :],
                                sim_sb[0:pn, i * HW:i * HW + HW])
            nc.vector.tensor_copy(ixf[0:pn, i:i + 1], ix8[0:pn, 0:1])
        with tc.tile_pool(name="ps_ir", bufs=2, space="PSUM") as ps_ir, \
             tc.tile_pool(name="ps_ib", bufs=1, space="PSUM") as ps_ib, \
             tc.tile_pool(name="ps_kt", bufs=2, space="PSUM") as ps_kt, \
             tc.tile_pool(name="ps_mt", bufs=1, space="PSUM") as ps_mt, \
             tc.tile_pool(name="cor2", bufs=2) as cor2:
            ir_sb = cor.tile([1, HW], F32, name="ir_sb")
            for i, (po, pn) in enumerate(PT):
                # one transpose per psum tile (start=True zeroes a whole bank)
                ir_ps = ps_ir.tile([1, 128], F32, name="ir_ps")
                nc.tensor.transpose(ir_ps[0:1, 0:pn], ixf[0:pn, i:i + 1],
                                    id_f[0:pn, 0:pn])
                nc.scalar.activation(ir_sb[0:1, po:po + pn], ir_ps[0:1, 0:pn],
                                     AF.Copy)
            ib_ps = ps_ib.tile([128, HW], F32, name="ib_ps")
            for (o, n) in _chunks(HW):
                nc.tensor.matmul(ib_ps[:, o:o + n], lhsT=onesr_sb[:],
                                 rhs=ir_sb[:, o:o + n], start=True, stop=True)
            ib_sb = cor.tile([128, HW], F32, name="ib_sb")
            nc.scalar.activation(ib_sb[:], ib_ps[:], AF.Copy)
            mt_ps = ps_mt.tile([128, HW], F32, name="mt_ps")
            for i, (po, pn) in enumerate(PT):
                S = cor2.tile([128, HW], BF16, name="S")
                nc.vector.tensor_scalar(
                    S[0:pn, :], ib_sb[0:pn, :], iota_sb[0:pn, :], float(po),
                    op0=ALU.subtract, op1=ALU.is_equal)
                kt_ps = ps_kt.tile([128, 128], BF16, name="kt_ps")
                nc.tensor.transpose(kt_ps[0:pn, :], kdT_bf[:, po:po + pn],
                                    id_b[:, :])
                kt_sb = cor2.tile([128, 128], BF16, name="kt_sb")
                nc.scalar.activation(kt_sb[0:pn, :], kt_ps[0:pn, :], AF.Copy)
                for (o, n) in _chunks(HW):
                    nc.tensor.matmul(mt_ps[:, o:o + n], lhsT=kt_sb[0:pn, :],
                                     rhs=S[0:pn, o:o + n],
                                     start=(i == 0), stop=(i == 6))
            nc.scalar.activation(matchT_sb[:], mt_ps[:], AF.Copy)
            nc.vector.tensor_copy(matchT_f8[:], mt_ps[:])
    nc.sync.dma_start(match_in[:].rearrange("(c p) -> c p", c=128), matchT_f8[:])
    # two-stage AllGather: 2-rank exchange then 4-rank gather — fewer ring
    # steps than a single 8-rank ring
    nc.gpsimd.collective_compute(
        "AllGather", ALU.bypass,
        replica_groups=[[0, 1], [2, 3], [4, 5], [6, 7]],
        ins=[match_in.opt()], outs=[match_mid.opt()])
    nc.gpsimd.collective_compute(
        "AllGather", ALU.bypass,
        replica_groups=[[0, 2, 4, 6], [1, 3, 5, 7]],
        ins=[match_mid.opt()], outs=[match_out.opt()])

    # =============== dense InfoNCE logits + global heads + queue ===============
    # Order inside this block is the PE-stream order for the whole tail:
    # logits matmuls feed the ScalarE exp chain ASAP; the global heads and
    # queue negatives (LDWEIGHTS-heavy, ScalarE-light) then fill the PE
    # while ScalarE grinds through ~50 us of exp.
    nc.scalar.dma_start(queueT_sb[:], g("queueT_d")[:])
    out_d = g("out_d")
    with tc.tile_pool(name="lg", bufs=1) as lgp, \
         tc.tile_pool(name="ps_lg", bufs=2, space="PSUM") as ps_lg:
        mall = lgp.tile([128, NT], F8, name="mall")
        nc.sync.dma_start(mall[:].rearrange("c (r p) -> c r p", r=8),
                          match_out[:].rearrange("(r c p) -> c r p", r=8, c=128))
        # positives: diag term = qd . matched (own block) - local data only,
        # so this runs during the AllGather wait
        nc.vector.tensor_copy(qdT_f8[:], qdT_bf[:])
        posm = lgp.tile([128, HW], F32, name="posm")
        nc.vector.tensor_mul(posm[:], qdT_bf[:], matchT_sb[:])
        fin_sb = lgp.tile([1, 32], F32, name="fin_sb")
        nc.vector.memset(fin_sb[:], 0.0)
        possum = lgp.tile([1, 1], F32, name="possum")
        with tc.tile_pool(name="ps_pos", bufs=1, space="PSUM") as ps_pos:
            pos_ps = ps_pos.tile([1, HW], F32, name="pos_ps")
            for (o, n) in _chunks(HW):
                nc.tensor.matmul(pos_ps[:, o:o + n], lhsT=onesc_sb[:],
                                 rhs=posm[:, o:o + n], start=True, stop=True)
            nc.vector.reduce_sum(possum[:], pos_ps[:], axis=mybir.AxisListType.X)
        zpart = lgp.tile([128, 56], F32, name="zpart")
        scr = lgp.tile([128, 1024], BF16, name="lg_scr")
        for i, (po, pn) in enumerate(PT):
            for ci, (co, cn) in enumerate(_chunks(NT, 1024)):
                lg_ps = ps_lg.tile([128, 1024], F32, name="lg_ps")
                for (o, n) in _chunks(cn):
                    nc.tensor.matmul(lg_ps[0:pn, o:o + n],
                                     lhsT=qdT_f8[:, po:po + pn],
                                     rhs=mall[:, co + o:co + o + n],
                                     start=True, stop=True)
                nc.scalar.activation(scr[0:pn, 0:cn], lg_ps[0:pn, 0:cn],
                                     AF.Exp, scale=ISC,
                                     accum_out=zpart[0:pn, i * 7 + ci:i * 7 + ci + 1])

        # ---- global heads (transposed orientation; fills PE under the exps)
        with tc.tile_pool(name="gap", bufs=1) as gap, \
             tc.tile_pool(name="gw1", bufs=2) as gw1, \
             tc.tile_pool(name="ggt", bufs=2) as ggt, \
             tc.tile_pool(name="ps_hgt", bufs=2, space="PSUM") as ps_hgt, \
             tc.tile_pool(name="ps_qg", bufs=1, space="PSUM") as ps_qg2, \
             tc.tile_pool(name="ps_g", bufs=1, space="PSUM") as ps_g:
            gaq = gap.tile([8, C], F32, name="gaq")
            gak = gap.tile([8, C], F32, name="gak")
            po = pool_out[:].rearrange("(r g x) -> r g x", r=8, g=2)
            nc.sync.dma_start(gaq[:], po[:, 0, :])
            nc.sync.dma_start(gak[:], po[:, 1, :])
            wg2_sb = load("wg2", (128, D), BF16)
            wg2m_sb = load("wg2m", (128, D), BF16)
            for br2, (ga, w1n, w2sb, b1c, b2, dstg) in enumerate([
                    (gaq, "wg1", wg2_sb, bg1_sb, bg2_sb, qgT_bf),
                    (gak, "wg1m", wg2m_sb, bg1m_sb, bg2m_sb, kgT_bf)]):
                gw1t = gw1.tile([128, CT * D], BF16, name="gw1t")
                nc.sync.dma_start(gw1t[:], g(w1n + "_d")[:])
                gqt_l = []
                for ct in range(CT):
                    gt_ps = ps_g.tile([128, 8], F32, name="gt_ps", tag="g8")
                    nc.tensor.transpose(gt_ps[:], ga[:, ct * 128:(ct + 1) * 128],
                                        id_f[0:8, 0:8])
                    gqt = ggt.tile([128, 8], BF16, name=f"gqt{ct}",
                                   tag=f"gqt{ct}")
                    nc.scalar.activation(gqt[:], gt_ps[:], AF.Copy,
                                         scale=1.0 / HW)
                    gqt_l.append(gqt)
                qg_ps = ps_qg2.tile([128, 8], F32, name="qg_ps", tag="qg")
                for dt in range(DT):
                    hgt_ps = ps_hgt.tile([128, 8], F32, name="hgt_ps",
                                         tag="hgt")
                    for ct in range(CT):
                        nc.tensor.matmul(
                            hgt_ps[:],
                            lhsT=gw1t[:, ct * D + dt * 128:
                                      ct * D + (dt + 1) * 128],
                            rhs=gqt_l[ct][:],
                            start=(ct == 0), stop=(ct == CT - 1))
                    hgt_sb = ggt.tile([128, 8], BF16, name="hgt_sb",
                                      tag="hgt_sb")
                    nc.scalar.activation(hgt_sb[:], hgt_ps[:], AF.Relu,
                                         bias=b1c[:, dt:dt + 1])
                    nc.tensor.matmul(qg_ps[:],
                                     lhsT=w2sb[:, dt * 128:(dt + 1) * 128],
                                     rhs=hgt_sb[:], start=(dt == 0),
                                     stop=(dt == DT - 1))
                qgT_f = gap.tile([128, 8], F32, name=f"qgT_f{br2}")
                nc.scalar.activation(qgT_f[:], qg_ps[:], AF.Identity,
                                     bias=b2[:])
                sqg = gap.tile([128, 8], BF16, name=f"sqg{br2}")
                nc.scalar.activation(sqg[:], qgT_f[:], AF.Square)
                ssg_ps = ps_g.tile([1, 8], F32, name="ssg_ps", tag="g8")
                nc.tensor.matmul(ssg_ps[:], lhsT=onescb_sb[:], rhs=sqg[:],
                                 start=True, stop=True)
                nrg = gap.tile([1, 8], F32, name=f"nrg{br2}")
                nc.vector.tensor_scalar_max(nrg[:], ssg_ps[:], 1e-12)
                nrg2 = gap.tile([1, 8], F32, name=f"nrg2{br2}")
                nc.scalar.activation(nrg2[:], nrg[:], AF.Ln)
                rng = gap.tile([1, 8], F32, name=f"rng{br2}")
                nc.scalar.activation(rng[:], nrg2[:], AF.Exp, scale=-0.5)
                rngb_ps = ps_g.tile([128, 8], F32, name="rngb_ps", tag="g8")
                nc.tensor.matmul(rngb_ps[:], lhsT=onesr_sb[:], rhs=rng[:],
                                 start=True, stop=True)
                nc.vector.tensor_mul(dstg[:], qgT_f[:], rngb_ps[:])
            # l_pos for every image (replicated)
            lpm = gap.tile([128, 8], F32, name="lpm")
            nc.vector.tensor_mul(lpm[:], qgT_bf[:], kgT_bf[:])
            lp_ps = ps_g.tile([1, 8], F32, name="lp_ps", tag="g8")
            nc.tensor.matmul(lp_ps[:], lhsT=onesc_sb[:], rhs=lpm[:],
                             start=True, stop=True)
            nc.scalar.activation(lpos_sb[:], lp_ps[:], AF.Copy)

            # ---- queue InfoNCE negatives (reuses the ghead psum pools so
            # the matmuls can fill the PE while the match-AllGather ends)
            qe_sb = gap.tile([128, 512], F32, name="qe_sb")
            for grp in range(8):
                qe_ps = ps_hgt.tile([128, 64], F32, name="qe_ps", tag="hgt")
                for j in range(8):
                    qt = grp * 8 + j
                    nc.tensor.matmul(qe_ps[:, j * 8:(j + 1) * 8],
                                     lhsT=queueT_sb[:, qt * 128:(qt + 1) * 128],
                                     rhs=qgT_bf[:], start=(j == 0),
                                     stop=(j == 7))
                nc.scalar.activation(qe_sb[:, grp * 64:(grp + 1) * 64],
                                     qe_ps[:], AF.Exp, scale=ISC)
            qs_ps = ps_qg2.tile([1, 512], F32, name="qs_ps", tag="qg")
            nc.tensor.matmul(qs_ps[:], lhsT=onesc_sb[:], rhs=qe_sb[:],
                             start=True, stop=True)
            qsums_sb = cst.tile([1, 8], F32, name="qsums_sb2")
            nc.vector.reduce_sum(qsums_sb[:],
                                 qs_ps[:].rearrange("p (t i) -> p i t", i=8),
                                 axis=mybir.AxisListType.X)
            nc.sync.dma_start(out_d[0:1, 0:8], qsums_sb[:])

        # ---- logsumexp finish + per-core partial outputs
        zs = lgp.tile([128, 7], F32, name="zs")
        lnz = lgp.tile([128, 7], F32, name="lnz")
        for i, (po, pn) in enumerate(PT):
            nc.vector.reduce_sum(zs[0:pn, i:i + 1],
                                 zpart[0:pn, i * 7:i * 7 + 7],
                                 axis=mybir.AxisListType.X)
            nc.scalar.activation(lnz[0:pn, i:i + 1], zs[0:pn, i:i + 1], AF.Ln)
        with tc.tile_pool(name="ps_f", bufs=2, space="PSUM") as ps_f:
            lz6_ps = ps_f.tile([1, 8], F32, name="lz6_ps")
            nc.tensor.matmul(lz6_ps[0:1, 0:6], lhsT=onesc_sb[:],
                             rhs=lnz[:, 0:6], start=True, stop=True)
            lz1_ps = ps_f.tile([1, 8], F32, name="lz1_ps")
            nc.tensor.matmul(lz1_ps[0:1, 0:1], lhsT=onesc_sb[0:16, :],
                             rhs=lnz[0:16, 6:7], start=True, stop=True)
            dsc = lgp.tile([1, 8], F32, name="dsc")
            nc.vector.memset(dsc[:], 0.0)
            nc.vector.tensor_copy(dsc[0:1, 0:6], lz6_ps[0:1, 0:6])
            nc.vector.tensor_copy(dsc[0:1, 6:7], lz1_ps[0:1, 0:1])
            nc.vector.tensor_scalar_mul(dsc[0:1, 7:8], possum[:], -ISC)
            # dense partial = sum(lnZ) - ISC*sum(pos); slot 8
            nc.vector.reduce_sum(fin_sb[0:1, 8:9], dsc[:],
                                 axis=mybir.AxisListType.X)
        # per-core partials; the final ~100-flop reduction happens on the
        # host after gathering all 8 cores' outputs
        nc.sync.dma_start(out_d[0:1, 8:9], fin_sb[0:1, 8:9])
        nc.sync.dma_start(out_d[0:1, 9:17], lpos_sb[:])


def _prep_inputs(inputs):
    fq = np.asarray(inputs["feat_q"], np.float32).reshape(B, HW, C)
    fk = np.asarray(inputs["feat_k"], np.float32).reshape(B, HW, C)

    def xT(x):  # (784, 1024) -> (128, 8*784) with [c, ct*784+p]
        return np.ascontiguousarray(
            x.reshape(HW, CT, 128).transpose(2, 1, 0).reshape(128, CT * HW)
        ).astype(BF)

    def w1tile(w):  # (1024, 2048) -> (16, 128, 1024) with [dt, c, ct*128+d]
        return np.ascontiguousarray(
            w.reshape(CT, 128, DT, 128).transpose(2, 1, 0, 3).reshape(DT, 128, C)
        ).astype(BF)

    def w2tile(w):  # (2048, 128) -> (128, 2048) with [c, dt*128+d]
        return np.ascontiguousarray(
            w.reshape(DT, 128, 128).transpose(1, 0, 2).reshape(128, D)
        ).astype(BF)

    def wg1tile(w):  # (1024, 2048) -> (128, 8*2048) with [c, ct*2048+d]
        return np.ascontiguousarray(
            w.reshape(CT, 128, D).transpose(1, 0, 2).reshape(128, CT * D)
        ).astype(BF)

    shared = {
        "wd1": w1tile(inputs["Wd1"]), "wd1m": w1tile(inputs["mWd1"]),
        "wd2": w2tile(inputs["Wd2"]), "wd2m": w2tile(inputs["mWd2"]),
        "wg1": wg1tile(inputs["Wg1"]), "wg1m": wg1tile(inputs["mWg1"]),
        "wg2": w2tile(inputs["Wg2"]), "wg2m": w2tile(inputs["mWg2"]),
        "bd1": np.ascontiguousarray(
            inputs["bd1"].reshape(DT, 128).T).astype(np.float32),
        "bd1m": np.ascontiguousarray(
            inputs["mbd1"].reshape(DT, 128).T).astype(np.float32),
        "bd2": np.asarray(inputs["bd2"], np.float32).reshape(128, 1),
        "bd2m": np.asarray(inputs["mbd2"], np.float32).reshape(128, 1),
        "bg1": np.ascontiguousarray(
            inputs["bg1"].reshape(DT, 128).T).astype(np.float32),
        "bg1m": np.ascontiguousarray(
            inputs["mbg1"].reshape(DT, 128).T).astype(np.float32),
        "bg2": np.asarray(inputs["bg2"], np.float32).reshape(128, 1),
        "bg2m": np.asarray(inputs["mbg2"], np.float32).reshape(128, 1),
        "iota": np.arange(128, dtype=np.float32).reshape(128, 1),
        "onesc": np.ones((128, 1), np.float32),
        "onesr": np.ones((1, 128), np.float32),
        "ones8": np.ones((1, 8), np.float32).astype(BF),
    }
    queue = np.asarray(inputs["queue"], np.float32)
    in_maps = []
    for c in range(N_CORES):
        m = dict(shared)
        m["xq"] = xT(fq[c])
        m["xk"] = xT(fk[c])
        m["queueT"] = np.ascontiguousarray(
            queue[c * QSH:(c + 1) * QSH].T).astype(BF)
        in_maps.append(m)
    return in_maps


_NC = None


def _get_nc():
    global _NC
    if _NC is None:
        _NC = _build()
    return _NC


def _host_combine(outs):
    """outs: [8, 32] per-core partials -> final scalar loss.

    Slots per core: [0:8] partial sum(exp(l_neg/tau)) over its queue shard
    (for each of the 8 images), [8] partial dense-InfoNCE sum over its 784
    rows, [9:17] l_pos per image (replicated on every core).
    """
    outs = np.asarray(outs, np.float64)
    qsums = outs[:, 0:8].sum(axis=0)
    dense_total = outs[:, 8].sum()
    lpos = outs[0, 9:17]
    lse = np.log(np.exp(ISC * lpos) + qsums)
    l_g = np.mean(lse - ISC * lpos)
    l_d = dense_total / NT
    return np.float32((1.0 - LAM) * l_g + LAM * l_d).reshape(())


def kernel(**inputs) -> np.ndarray:
    nc = _get_nc()
    in_maps = _prep_inputs(inputs)
    res = bass_utils.run_bass_kernel_spmd(nc, in_maps,
                                          core_ids=list(range(N_CORES)))
    outs = np.stack([res.results[c]["out"].reshape(32)
                     for c in range(N_CORES)])
    return _host_combine(outs)



# revision 14
# speedup vs baseline: 1.5517x; 1.5517x over previous
"""DenseCL loss kernel for 8 TRN2 NeuronCores (v2: fp8 DoubleRow + column-
sharded dense-InfoNCE logits).

Sharding: core c owns image c (dense head + correspondence + matched keys),
queue rows [c*8192, (c+1)*8192), and the COLUMN shard of the flat dense
logits: core c computes partial exp-sums over its own 784 matched-key
columns for ALL 6272 q rows; the host sums the per-core z partials.  The
only critical-path collective is a single early AllGather of the fp8
q_d (+ q_g) launched right after the q branch, hidden under the k branch.

Dense/global head matmuls run in fp8e4 with DoubleRow (2 contraction rows
per PE cell); weights are pre-scaled x64 on the host, the 1/64 folds into
the activation scale.  End-to-end fp8 rel-err vs the fp32 reference is
~5e-4 (validated in numpy), far under the 2e-2 gate.
"""
import sys

if "/opt/trn_rl_repo" not in sys.path:
    sys.path.insert(0, "/opt/trn_rl_repo")

import numpy as np
import ml_dtypes

import concourse.bass as bass
import concourse.bacc as bacc
import concourse.mybir as mybir
import concourse.tile as tile
from concourse import bass_utils, masks

F8np = ml_dtypes.float8_e4m3     # TRN FP8_EXP4-compatible (bias 7, max 240)
BFnp = ml_dtypes.bfloat16
F32 = mybir.dt.float32
BF16 = mybir.dt.bfloat16
F8 = mybir.dt.float8e4
DR = mybir.MatmulPerfMode.DoubleRow

N_CORES = 8
B, HW, C, D, P, Q = 8, 784, 1024, 2048, 128, 65536
QSH = Q // N_CORES          # 8192 queue rows per core
CT, DT = C // 128, D // 128  # 8, 16
NT = B * HW                 # 6272 total dense rows
RT = NT // 128              # 49 flat q-row tiles
TAU = 0.2
LAM = 0.5
ISC = 1.0 / TAU             # 5.0
AF = mybir.ActivationFunctionType
ALU = mybir.AluOpType

# 784 = 6*128 + 16 partition tiles (correspondence)
PT = [(i * 128, min(128, HW - i * 128)) for i in range(7)]
CH = [(0, 512), (512, HW - 512)]   # free-dim chunks of 784


def _patch_act_tables():
    """Force every activation we use onto the natural_log_exp_and_others
    table set so the kernel needs exactly one ACT_TABLE_LOAD."""
    import concourse.bacc as bacc_mod
    if getattr(bacc_mod, "_act_tables_patched", False):
        return
    from concourse import hw_specs
    orig = hw_specs.get_activation_tables
    ours = {AF.Exp, AF.Ln, AF.Relu, AF.Identity, AF.Copy, AF.Square}
    keep = "natural_log_exp_and_others"

    def patched(arch):
        tabs = orig(arch)
        assert keep in tabs and ours <= tabs[keep]
        return {name: (fns if name == keep else fns - ours)
                for name, fns in tabs.items()}

    bacc_mod.get_activation_tables = patched
    bacc_mod._act_tables_patched = True


def _build(do_compile=True):
    _patch_act_tables()
    nc = bacc.Bacc("TRN2", target_bir_lowering=False, debug=False,
                   num_devices=N_CORES)

    def inp(name, shape, dt):
        return nc.dram_tensor(name, list(shape), dt, kind="ExternalInput")

    env = {}
    env["xq_d"] = inp("xq", (128, CT * HW), F8)    # [c, ct*784+p]
    env["xk_d"] = inp("xk", (128, CT * HW), F8)
    env["wd1_d"] = inp("wd1", (DT, 128, C), F8)    # [dt, c, ct*128+d] x64
    env["wd1m_d"] = inp("wd1m", (DT, 128, C), F8)
    env["wd2_d"] = inp("wd2", (128, D), F8)        # [d, dt*128+p] x64
    env["wd2m_d"] = inp("wd2m", (128, D), F8)
    env["wg1_d"] = inp("wg1", (128, CT * D), F8)   # [c, ct*2048+d] x64
    env["wg1m_d"] = inp("wg1m", (128, CT * D), F8)
    env["wg2_d"] = inp("wg2", (128, D), F8)        # like wd2, x64
    env["wg2m_d"] = inp("wg2m", (128, D), F8)
    env["bd1_d"] = inp("bd1", (128, DT), F32)      # [r, dt] = bd1[dt*128+r]
    env["bd1m_d"] = inp("bd1m", (128, DT), F32)
    env["bd2_d"] = inp("bd2", (128, 1), F32)
    env["bd2m_d"] = inp("bd2m", (128, 1), F32)
    env["bg1r_d"] = inp("bg1r", (1, D), BF16)      # bg1 x4096 (bias row)
    env["bg1mr_d"] = inp("bg1mr", (1, D), BF16)
    env["bg2_d"] = inp("bg2", (128, 1), F32)
    env["bg2m_d"] = inp("bg2m", (128, 1), F32)
    env["queueT_d"] = inp("queueT", (128, QSH), F8)  # 8*queue[c0+j, ch]
    env["iota_d"] = inp("iota", (128, 1), F32)
    env["onesc_d"] = inp("onesc", (128, 1), F32)
    env["onesr_d"] = inp("onesr", (1, 128), F32)
    env["ones8_d"] = inp("ones8", (1, 8), BF16)

    env["outz_d"] = nc.dram_tensor("outz", [128, RT], F32,
                                   kind="ExternalOutput")
    env["outs_d"] = nc.dram_tensor("outs", [1, 16], F32,
                                   kind="ExternalOutput")

    with tile.TileContext(nc) as tc:
        with tc.tile_pool(name="dramp", bufs=1, space="DRAM") as dpool:
            env["ag_in"] = dpool.tile([128 * 785], F8, name="ag_in")
            env["ag_out"] = dpool.tile([N_CORES * 128 * 785], F8,
                                       name="ag_out", addr_space="Shared")
            with tc.tile_pool(name="cst", bufs=1) as cst:
                _body(nc, tc, env, cst)
    if do_compile:
        nc.compile()
    return nc


def _dense_branch(nc, tc, env, cst, br, pools, tiles):
    """One dense-head branch (q: br=0, k: br=1) -> normalized [128, HW]."""
    g = lambda k: env[k]
    sfx = "" if br == 0 else "m"
    x_sb = tiles["xq8" if br == 0 else "xk8"]
    w1_d = g("wd1" + sfx + "_d")
    w2_sb = tiles["wd2" + sfx]
    b1 = tiles["bd1" + sfx]
    b2 = tiles["bd2" + sfx]
    dst_bf = tiles["qdT_bf" if br == 0 else "kdT_bf"]
    w1p, hp, l2s, ps_h, ps_m = (pools["w1p"], pools["hp"], pools["l2s"],
                                pools["ps_h"], pools["ps_m"])

    xv = x_sb[:].rearrange("c (t p) -> c t p", t=CT)
    qd_ps = ps_m.tile([128, HW], F32, name=f"qd_ps{br}", tag="m")
    hq = None
    for dt in range(DT):
        w1t = w1p.tile([128, C], F8, name=f"w1t{br}")
        nc.sync.dma_start(w1t[:], w1_d[dt, :, :])
        h_ps = ps_h.tile([128, HW], F32, name="h_ps", tag="h")
        wv = w1t[:].rearrange("c (t d) -> c t d", t=CT)
        for kp in range(CT // 2):
            for (o, n) in CH:
                nc.tensor.matmul(
                    h_ps[:, o:o + n],
                    lhsT=wv[:, 2 * kp:2 * kp + 2, :],
                    rhs=xv[:, 2 * kp:2 * kp + 2, o:o + n],
                    start=(kp == 0), stop=(kp == CT // 2 - 1),
                    perf_mode=DR)
        if dt % 2 == 0:
            hq = hp.tile([128, 2 * HW], F8, name=f"hq{br}")
        nc.scalar.activation(hq[:, (dt % 2) * HW:(dt % 2 + 1) * HW],
                             h_ps[:], AF.Relu, bias=b1[:, dt:dt + 1],
                             scale=1.0 / 64.0)
        if br == 0:
            # stagger the k-branch / tail input DMAs on the scalar ring so
            # they don't compete with the q-critical loads
            if dt == 2:
                nc.scalar.dma_start(tiles["xk8"][:], g("xk_d")[:])
            elif dt == 6:
                nc.scalar.dma_start(tiles["wd2m"][:], g("wd2m_d")[:])
                nc.scalar.dma_start(tiles["wg1m"][:], g("wg1m_d")[:])
            elif dt == 10:
                nc.scalar.dma_start(tiles["queueT8"][:], g("queueT_d")[:])
                nc.scalar.dma_start(tiles["wg2m"][:], g("wg2m_d")[:])
        if dt % 2 == 1:
            dp = dt // 2
            w2v = w2_sb[:].rearrange("c (t d) -> c t d", t=DT)
            hv = hq[:].rearrange("c (j p) -> c j p", j=2)
            for (o, n) in CH:
                nc.tensor.matmul(
                    qd_ps[:, o:o + n],
                    lhsT=w2v[:, 2 * dp:2 * dp + 2, :],
                    rhs=hv[:, :, o:o + n],
                    start=(dp == 0), stop=(dp == DT // 2 - 1),
                    perf_mode=DR)

    # bias + l2 normalize along channels (partition dim)
    qdT_f = l2s.tile([128, HW], F32, name=f"qdT_f{br}")
    nc.scalar.activation(qdT_f[:], qd_ps[:], AF.Identity, bias=b2[:],
                         scale=1.0 / 64.0)
    sq = l2s.tile([128, HW], BF16, name=f"sq{br}")
    nc.scalar.activation(sq[:], qdT_f[:], AF.Square)
    ssq_ps = ps_m.tile([1, HW], F32, name=f"ssq{br}", tag="m")
    for (o, n) in CH:
        nc.tensor.matmul(ssq_ps[:, o:o + n], lhsT=tiles["onescb"][:],
                         rhs=sq[:, o:o + n], start=True, stop=True)
    nrm = l2s.tile([1, HW], F32, name=f"nrm{br}")
    nc.vector.tensor_scalar_max(nrm[:], ssq_ps[:], 1e-12)
    nrm2 = l2s.tile([1, HW], F32, name=f"nrm2{br}")
    nc.scalar.activation(nrm2[:], nrm[:], AF.Ln)
    rn = l2s.tile([1, HW], F32, name=f"rn{br}")
    nc.scalar.activation(rn[:], nrm2[:], AF.Exp, scale=-0.5)
    rnb_ps = ps_m.tile([128, HW], F32, name=f"rnb{br}", tag="m")
    for (o, n) in CH:
        nc.tensor.matmul(rnb_ps[:, o:o + n], lhsT=tiles["onesr"][:],
                         rhs=rn[:, o:o + n], start=True, stop=True)
    nc.vector.tensor_mul(dst_bf[:], qdT_f[:], rnb_ps[:])
    return dst_bf


def _ghead_branch(nc, tc, env, cst, br, pools, tiles):
    """Global head for the core's own image (q: br=0, k: br=1)."""
    g = lambda k: env[k]
    sfx = "" if br == 0 else "m"
    x_sb = tiles["xq8" if br == 0 else "xk8"]
    w1_sb = tiles["wg1" + sfx]
    w2_sb = tiles["wg2" + sfx]
    b1r = tiles["bg1r" if br == 0 else "bg1mr"]
    b2 = tiles["bg2" + sfx]
    dst_bf = tiles["qgT_bf" if br == 0 else "kgT_bf"]
    gp, ps_m = pools["gp"], pools["ps_m"]
    ones1 = tiles["ones8"][0:1, 0:1]

    # pooling: g*64 staged at stride 16 for the DoubleRow stationary
    gsum = gp.tile([128, CT], F32, name=f"gsum{br}")
    for ct in range(CT):
        pscr = pools["pscr"].tile([128, HW], F8, name="pscr")
        nc.vector.tensor_scalar(pscr[:], x_sb[:, ct * HW:(ct + 1) * HW],
                                1.0, None, op0=ALU.mult, op1=ALU.add,
                                accum_out=gsum[:, ct:ct + 1])
    gqt8 = gp.tile([128, CT * 16], F8, name=f"gqt8{br}")
    gq_v = gqt8[:].rearrange("c (t s) -> c t s", s=16)
    nc.vector.tensor_scalar_mul(gq_v[:, :, 0:1], gsum[:], 64.0 / HW)

    # L1: h_g[1, 2048] = (g*64) @ (Wg1*64) / 4096 + bg1, in 512-chunks
    hgb = gp.tile([1, D], BF16, name=f"hgb{br}")
    w1v = w1_sb[:].rearrange("c (t d) -> c t d", t=CT)
    for ch in range(4):
        hg_ps = ps_m.tile([1, 512], F32, name=f"hg{br}", tag="m")
        for kp in range(CT // 2):
            nc.tensor.matmul(
                hg_ps[:], lhsT=gq_v[:, 2 * kp:2 * kp + 2, 0:1],
                rhs=w1v[:, 2 * kp:2 * kp + 2, ch * 512:(ch + 1) * 512],
                start=(kp == 0), stop=False, perf_mode=DR)
        nc.tensor.matmul(hg_ps[:], lhsT=ones1,
                         rhs=b1r[0:1, ch * 512:(ch + 1) * 512],
                         start=False, stop=True)
        nc.scalar.activation(hgb[0:1, ch * 512:(ch + 1) * 512], hg_ps[:],
                             AF.Relu, scale=1.0 / 4096.0)
    # transpose h_g -> [128, DT] via K=1 matmuls, then fp8 at stride 16
    hgt_ps = ps_m.tile([128, DT], F32, name=f"hgt{br}", tag="m")
    for dt in range(DT):
        nc.tensor.matmul(hgt_ps[:, dt:dt + 1],
                         lhsT=hgb[0:1, dt * 128:(dt + 1) * 128],
                         rhs=ones1, start=(dt == 0), stop=(dt == DT - 1))
    hgt8 = gp.tile([128, DT * 16], F8, name=f"hgt8{br}")
    hgt_v = hgt8[:].rearrange("c (t s) -> c t s", s=16)
    nc.scalar.activation(hgt_v[:, :, 0:1], hgt_ps[:], AF.Copy)
    # L2: q_g[128, 1]
    qg_ps = ps_m.tile([128, 1], F32, name=f"qg{br}", tag="m")
    w2v = w2_sb[:].rearrange("c (t d) -> c t d", t=DT)
    for dp in range(DT // 2):
        nc.tensor.matmul(qg_ps[:], lhsT=w2v[:, 2 * dp:2 * dp + 2, :],
                         rhs=hgt_v[:, 2 * dp:2 * dp + 2, 0:1],
                         start=(dp == 0), stop=(dp == DT // 2 - 1),
                         perf_mode=DR)
    qgT_f = gp.tile([128, 1], F32, name=f"qgT_f{br}")
    nc.scalar.activation(qgT_f[:], qg_ps[:], AF.Identity, bias=b2[:],
                         scale=1.0 / 64.0)
    sqg = gp.tile([128, 1], BF16, name=f"sqg{br}")
    nc.scalar.activation(sqg[:], qgT_f[:], AF.Square)
    ssg_ps = ps_m.tile([1, 1], F32, name=f"ssg{br}", tag="m")
    nc.tensor.matmul(ssg_ps[:], lhsT=tiles["onescb"][:], rhs=sqg[:],
                     start=True, stop=True)
    nrg = gp.tile([1, 1], F32, name=f"nrg{br}")
    nc.vector.tensor_scalar_max(nrg[:], ssg_ps[:], 1e-12)
    nrg2 = gp.tile([1, 1], F32, name=f"nrg2{br}")
    nc.scalar.activation(nrg2[:], nrg[:], AF.Ln)
    rng = gp.tile([1, 1], F32, name=f"rng{br}")
    nc.scalar.activation(rng[:], nrg2[:], AF.Exp, scale=-0.5)
    rngb_ps = ps_m.tile([128, 1], F32, name=f"rngb{br}", tag="m")
    nc.tensor.matmul(rngb_ps[:], lhsT=tiles["onesr"][:], rhs=rng[:],
                     start=True, stop=True)
    nc.vector.tensor_mul(dst_bf[:], qgT_f[:], rngb_ps[:])
    return dst_bf


def _body(nc, tc, env, cst):
    g = lambda k: env[k]
    tiles = {}

    # ---------------- inputs into SBUF ----------------
    tiles["xq8"] = cst.tile([128, CT * HW], F8, name="xq8")
    nc.sync.dma_start(tiles["xq8"][:], g("xq_d")[:])
    # scalar ring: q-side weights first
    for nm in ("wd2", "wg2", "wg1"):
        t = cst.tile([128, {"wd2": D, "wg2": D, "wg1": CT * D}[nm]], F8,
                     name=nm)
        nc.scalar.dma_start(t[:], g(nm + "_d")[:])
        tiles[nm] = t
    # k-side tiles (DMAs staggered inside the q loop)
    tiles["xk8"] = cst.tile([128, CT * HW], F8, name="xk8")
    tiles["wd2m"] = cst.tile([128, D], F8, name="wd2m")
    tiles["wg1m"] = cst.tile([128, CT * D], F8, name="wg1m")
    tiles["wg2m"] = cst.tile([128, D], F8, name="wg2m")
    tiles["queueT8"] = cst.tile([128, QSH], F8, name="queueT8")
    # small consts on the gpsimd ring
    for nm, shp, dt in (("iota", (128, 1), F32), ("onesc", (128, 1), F32),
                        ("onesr", (1, 128), F32), ("ones8", (1, 8), BF16),
                        ("bd1", (128, DT), F32), ("bd1m", (128, DT), F32),
                        ("bd2", (128, 1), F32), ("bd2m", (128, 1), F32),
                        ("bg1r", (1, D), BF16), ("bg1mr", (1, D), BF16),
                        ("bg2", (128, 1), F32), ("bg2m", (128, 1), F32)):
        t = cst.tile(list(shp), dt, name=nm)
        nc.gpsimd.dma_start(t[:], g(nm + "_d")[:])
        tiles[nm] = t
    tiles["onescb"] = cst.tile([128, 1], BF16, name="onescb")
    nc.vector.tensor_copy(tiles["onescb"][:], tiles["onesc"][:])
    id_f = cst.tile([128, 128], F32, name="id_f")
    masks.make_identity(nc, id_f[:])
    id_b = cst.tile([128, 128], BF16, name="id_b")
    masks.make_identity(nc, id_b[:])

    # long-lived results
    for nm, shp, dt in (("qdT_bf", (128, HW), BF16),
                        ("kdT_bf", (128, HW), BF16),
                        ("qgT_bf", (128, 1), BF16),
                        ("kgT_bf", (128, 1), BF16),
                        ("qd8s", (128, 785), F8),
                        ("qall", (128, NT), F8),
                        ("qgall", (128, 8), F8),
                        ("matchT", (128, HW), BF16),
                        ("matchT8", (128, HW), F8),
                        ("zpart", (128, RT), F32),
                        ("fin", (1, 16), F32)):
        tiles[nm] = cst.tile(list(shp), dt, name=nm)
    nc.vector.memset(tiles["fin"][:], 0.0)

    pools = {}
    with tc.tile_pool(name="w1p", bufs=8) as pools["w1p"], \
         tc.tile_pool(name="hp", bufs=2) as pools["hp"], \
         tc.tile_pool(name="l2s", bufs=2) as pools["l2s"], \
         tc.tile_pool(name="gp", bufs=1) as pools["gp"], \
         tc.tile_pool(name="pscr", bufs=2) as pools["pscr"], \
         tc.tile_pool(name="ps_h", bufs=2, space="PSUM") as pools["ps_h"], \
         tc.tile_pool(name="ps_m", bufs=2, space="PSUM") as pools["ps_m"]:

        # ========== q branch + its global head, then the AllGather ==========
        _dense_branch(nc, tc, env, cst, 0, pools, tiles)
        nc.vector.tensor_scalar_mul(tiles["qd8s"][:, 0:HW],
                                    tiles["qdT_bf"][:], 8.0)
        _ghead_branch(nc, tc, env, cst, 0, pools, tiles)
        nc.vector.tensor_scalar_mul(tiles["qd8s"][:, HW:HW + 1],
                                    tiles["qgT_bf"][:], 8.0)
        ag_in, ag_out = g("ag_in"), g("ag_out")
        nc.gpsimd.dma_start(ag_in[:].rearrange("(c p) -> c p", c=128),
                            tiles["qd8s"][:])
        nc.gpsimd.collective_compute(
            "AllGather", ALU.bypass, replica_groups=[list(range(N_CORES))],
            ins=[ag_in.opt()], outs=[ag_out.opt()])

        # ========== k branch + its global head ==========
        _dense_branch(nc, tc, env, cst, 1, pools, tiles)
        _ghead_branch(nc, tc, env, cst, 1, pools, tiles)

        # lpos = q_g . k_g (own image)
        lpm = pools["gp"].tile([128, 1], F32, name="lpm")
        nc.vector.tensor_mul(lpm[:], tiles["qgT_bf"][:], tiles["kgT_bf"][:])
        lp_ps = pools["ps_m"].tile([1, 1], F32, name="lp_ps", tag="m")
        nc.tensor.matmul(lp_ps[:], lhsT=tiles["onesc"][:], rhs=lpm[:],
                         start=True, stop=True)
        nc.vector.tensor_copy(tiles["fin"][0:1, 1:2], lp_ps[:])

        # ========== correspondence (own image, bf16) ==========
        qdT_bf, kdT_bf = tiles["qdT_bf"], tiles["kdT_bf"]
        with tc.tile_pool(name="cor", bufs=1) as cor, \
             tc.tile_pool(name="cor2", bufs=2) as cor2:
            sim_sb = cor.tile([128, 7 * HW], BF16, name="sim_sb")
            for i, (po, pn) in enumerate(PT):
                s_ps = pools["ps_h"].tile([128, HW], F32, name="s_ps",
                                          tag="h")
                for (o, n) in CH:
                    nc.tensor.matmul(s_ps[0:pn, o:o + n],
                                     lhsT=qdT_bf[:, po:po + pn],
                                     rhs=kdT_bf[:, o:o + n],
                                     start=True, stop=True)
                nc.scalar.activation(sim_sb[0:pn, i * HW:i * HW + HW],
                                     s_ps[0:pn, :], AF.Copy)
            mx8 = cor.tile([128, 8], F32, name="mx8")
            ix8 = cor.tile([128, 8], mybir.dt.uint32, name="ix8")
            ixf = cor.tile([128, 7], F32, name="ixf")
            for i, (po, pn) in enumerate(PT):
                nc.vector.max(mx8[0:pn, :], sim_sb[0:pn, i * HW:i * HW + HW])
                nc.vector.max_index(ix8[0:pn, :], mx8[0:pn, :],
                                    sim_sb[0:pn, i * HW:i * HW + HW])
                nc.vector.tensor_copy(ixf[0:pn, i:i + 1], ix8[0:pn, 0:1])
            ir_sb = cor.tile([1, HW], F32, name="ir_sb")
            for i, (po, pn) in enumerate(PT):
                ir_ps = pools["ps_m"].tile([1, 128], F32, name="ir_ps",
                                           tag="m")
                nc.tensor.transpose(ir_ps[0:1, 0:pn], ixf[0:pn, i:i + 1],
                                    id_f[0:pn, 0:pn])
                nc.scalar.activation(ir_sb[0:1, po:po + pn],
                                     ir_ps[0:1, 0:pn], AF.Copy)
            ib_ps = pools["ps_m"].tile([128, HW], F32, name="ib_ps", tag="m")
            for (o, n) in CH:
                nc.tensor.matmul(ib_ps[:, o:o + n], lhsT=tiles["onesr"][:],
                                 rhs=ir_sb[:, o:o + n], start=True, stop=True)
            ib_sb = cor.tile([128, HW], F32, name="ib_sb")
            nc.scalar.activation(ib_sb[:], ib_ps[:], AF.Copy)
            # gather matched keys via one-hot matmuls; mt_ps stays resident
            # in ps_m while kt transposes rotate through ps_h
            mt_ps = pools["ps_m"].tile([128, HW], F32, name="mt_ps", tag="m")
            for i, (po, pn) in enumerate(PT):
                S = cor2.tile([128, HW], BF16, name="S")
                nc.vector.tensor_scalar(
                    S[0:pn, :], ib_sb[0:pn, :], tiles["iota"][0:pn, :],
                    float(po), op0=ALU.subtract, op1=ALU.is_equal)
                kt_ps = pools["ps_h"].tile([128, 128], BF16, name="kt_ps",
                                           tag="h")
                nc.tensor.transpose(kt_ps[0:pn, :], kdT_bf[:, po:po + pn],
                                    id_b[:, :])
                kt_sb = cor2.tile([128, 128], BF16, name="kt_sb")
                nc.scalar.activation(kt_sb[0:pn, :], kt_ps[0:pn, :], AF.Copy)
                for (o, n) in CH:
                    nc.tensor.matmul(mt_ps[:, o:o + n], lhsT=kt_sb[0:pn, :],
                                     rhs=S[0:pn, o:o + n],
                                     start=(i == 0), stop=(i == 6))
            nc.scalar.activation(tiles["matchT"][:], mt_ps[:], AF.Copy)
            nc.vector.tensor_scalar_mul(tiles["matchT8"][:], mt_ps[:], 8.0)

            # positives: diag = qd . matched (own rows), summed
            posm = cor.tile([128, HW], F32, name="posm")
            nc.vector.tensor_mul(posm[:], qdT_bf[:], tiles["matchT"][:])
            pos_ps = pools["ps_m"].tile([1, HW], F32, name="pos_ps", tag="m")
            for (o, n) in CH:
                nc.tensor.matmul(pos_ps[:, o:o + n], lhsT=tiles["onesc"][:],
                                 rhs=posm[:, o:o + n], start=True, stop=True)
            nc.vector.reduce_sum(tiles["fin"][0:1, 0:1], pos_ps[:],
                                 axis=mybir.AxisListType.X)

        # ========== gathered q + queue negatives + dense logits ==========
        agv = ag_out[:].rearrange("(r c p) -> c r p", r=N_CORES, c=128)
        nc.sync.dma_start(
            tiles["qgall"][:].rearrange("c (r p) -> c r p", p=1),
            agv[:, :, HW:HW + 1])
        nc.sync.dma_start(
            tiles["qall"][:].rearrange("c (r p) -> c r p", r=N_CORES),
            agv[:, :, 0:HW])

        with tc.tile_pool(name="qes", bufs=2) as qes, \
             tc.tile_pool(name="escr", bufs=2) as escr:
            # queue InfoNCE negatives: own 8192-row shard vs all 8 q_g
            qe_sb = cst.tile([128, 512], BF16, name="qe_sb")
            for grp in range(8):
                qe_ps = pools["ps_m"].tile([128, 64], F32, name="qe_ps",
                                           tag="m")
                for j in range(8):
                    qt = grp * 8 + j
                    nc.tensor.matmul(
                        qe_ps[:, j * 8:(j + 1) * 8],
                        lhsT=tiles["queueT8"][:, qt * 128:(qt + 1) * 128],
                        rhs=tiles["qgall"][:], start=(j == 0), stop=(j == 7))
                nc.scalar.activation(qe_sb[:, grp * 64:(grp + 1) * 64],
                                     qe_ps[:], AF.Exp, scale=ISC / 64.0)
            qs_ps = pools["ps_m"].tile([1, 512], F32, name="qs_ps", tag="m")
            nc.tensor.matmul(qs_ps[:], lhsT=tiles["onescb"][:], rhs=qe_sb[:],
                             start=True, stop=True)
            nc.vector.reduce_sum(tiles["fin"][0:1, 2:10],
                                 qs_ps[:].rearrange("p (t i) -> p i t", i=8),
                                 axis=mybir.AxisListType.X)

            # dense logits, column shard: all 6272 q rows x own 784 keys
            for t in range(RT):
                lg_ps = pools["ps_h"].tile([128, HW], F32, name="lg_ps",
                                           tag="h")
                for (o, n) in CH:
                    nc.tensor.matmul(
                        lg_ps[:, o:o + n],
                        lhsT=tiles["qall"][:, t * 128:(t + 1) * 128],
                        rhs=tiles["matchT8"][:, o:o + n],
                        start=True, stop=True)
                es = escr.tile([128, HW], BF16, name="es")
                nc.scalar.activation(es[:], lg_ps[:], AF.Exp,
                                     scale=ISC / 64.0,
                                     accum_out=tiles["zpart"][:, t:t + 1])

        nc.sync.dma_start(g("outz_d")[:], tiles["zpart"][:])
        nc.sync.dma_start(g("outs_d")[:], tiles["fin"][:])


def _prep_inputs(inputs):
    fq = np.asarray(inputs["feat_q"], np.float32).reshape(B, HW, C)
    fk = np.asarray(inputs["feat_k"], np.float32).reshape(B, HW, C)

    def xT8(x):  # (784, 1024) -> (128, 8*784) f8 with [c, ct*784+p]
        return np.ascontiguousarray(
            x.reshape(HW, CT, 128).transpose(2, 1, 0).reshape(128, CT * HW)
        ).astype(F8np)

    def w1tile(w):  # (1024, 2048) -> (16, 128, 1024) f8 x64
        return np.ascontiguousarray(
            (w * 64.0).reshape(CT, 128, DT, 128).transpose(2, 1, 0, 3)
            .reshape(DT, 128, C)).astype(F8np)

    def w2tile(w):  # (2048, 128) -> (128, 2048) f8 x64
        return np.ascontiguousarray(
            (w * 64.0).reshape(DT, 128, 128).transpose(1, 0, 2)
            .reshape(128, D)).astype(F8np)

    def wg1tile(w):  # (1024, 2048) -> (128, 8*2048) f8 x64
        return np.ascontiguousarray(
            (w * 64.0).reshape(CT, 128, D).transpose(1, 0, 2)
            .reshape(128, CT * D)).astype(F8np)

    shared = {
        "wd1": w1tile(inputs["Wd1"]), "wd1m": w1tile(inputs["mWd1"]),
        "wd2": w2tile(inputs["Wd2"]), "wd2m": w2tile(inputs["mWd2"]),
        "wg1": wg1tile(inputs["Wg1"]), "wg1m": wg1tile(inputs["mWg1"]),
        "wg2": w2tile(inputs["Wg2"]), "wg2m": w2tile(inputs["mWg2"]),
        "bd1": np.ascontiguousarray(
            np.asarray(inputs["bd1"], np.float32).reshape(DT, 128).T),
        "bd1m": np.ascontiguousarray(
            np.asarray(inputs["mbd1"], np.float32).reshape(DT, 128).T),
        "bd2": np.asarray(inputs["bd2"], np.float32).reshape(128, 1),
        "bd2m": np.asarray(inputs["mbd2"], np.float32).reshape(128, 1),
        "bg1r": (np.asarray(inputs["bg1"], np.float32) * 4096.0
                 ).reshape(1, D).astype(BFnp),
        "bg1mr": (np.asarray(inputs["mbg1"], np.float32) * 4096.0
                  ).reshape(1, D).astype(BFnp),
        "bg2": np.asarray(inputs["bg2"], np.float32).reshape(128, 1),
        "bg2m": np.asarray(inputs["mbg2"], np.float32).reshape(128, 1),
        "iota": np.arange(128, dtype=np.float32).reshape(128, 1),
        "onesc": np.ones((128, 1), np.float32),
        "onesr": np.ones((1, 128), np.float32),
        "ones8": np.ones((1, 8), np.float32).astype(BFnp),
    }
    queue = np.asarray(inputs["queue"], np.float32)
    in_maps = []
    for c in range(N_CORES):
        m = dict(shared)
        m["xq"] = xT8(fq[c])
        m["xk"] = xT8(fk[c])
        m["queueT"] = np.ascontiguousarray(
            (queue[c * QSH:(c + 1) * QSH] * 8.0).T).astype(F8np)
        in_maps.append(m)
    return in_maps


_NC = None


def _get_nc():
    global _NC
    if _NC is None:
        _NC = _build()
    return _NC


def _host_combine(outz, outs):
    """outz: [8][128, 49] z-partials; outs: [8][1, 16] scalars.

    outs slots: [0] sum(qd.matched) over own rows, [1] own-image lpos,
    [2:10] partial sum(exp(l_neg/tau)) per image over the core's queue
    shard.  Dense z row r=t*128+p lives at outz[:, p, t].
    """
    outz = np.asarray(outz, np.float64)   # [8, 128, 49]
    outs = np.asarray(outs, np.float64)   # [8, 16]
    z = outz.sum(axis=0)                  # [128, 49]
    zrows = z.T.reshape(-1)               # row r = t*128+p
    pos_total = outs[:, 0].sum()
    l_d = (np.log(zrows).sum() - ISC * pos_total) / NT
    zq = outs[:, 2:10].sum(axis=0)        # [8]
    lpos = outs[np.arange(8), 1]          # core c owns image c
    lse = np.log(zq + np.exp(ISC * lpos))
    l_g = np.mean(lse - ISC * lpos)
    return np.float32((1.0 - LAM) * l_g + LAM * l_d).reshape(())


def kernel(**inputs) -> np.ndarray:
    nc = _get_nc()
    in_maps = _prep_inputs(inputs)
    res = bass_utils.run_bass_kernel_spmd(nc, in_maps,
                                          core_ids=list(range(N_CORES)))
    outz = np.stack([res.results[c]["outz"] for c in range(N_CORES)])
    outs = np.stack([res.results[c]["outs"].reshape(16)
                     for c in range(N_CORES)])
    return _host_combine(outz, outs)


# revision 23
# speedup vs baseline: 1.5787x; 1.0174x over previous
"""DenseCL loss kernel for 8 TRN2 NeuronCores (v2: fp8 DoubleRow + column-
sharded dense-InfoNCE logits).

Sharding: core c owns image c (dense head + correspondence + matched keys),
queue rows [c*8192, (c+1)*8192), and the COLUMN shard of the flat dense
logits: core c computes partial exp-sums over its own 784 matched-key
columns for ALL 6272 q rows; the host sums the per-core z partials.  The
only critical-path collective is a single early AllGather of the fp8
q_d (+ q_g) launched right after the q branch, hidden under the k branch.

Dense/global head matmuls run in fp8e4 with DoubleRow (2 contraction rows
per PE cell); weights are pre-scaled x64 on the host, the 1/64 folds into
the activation scale.  End-to-end fp8 rel-err vs the fp32 reference is
~5e-4 (validated in numpy), far under the 2e-2 gate.
"""
import sys

if "/opt/trn_rl_repo" not in sys.path:
    sys.path.insert(0, "/opt/trn_rl_repo")

import numpy as np
import ml_dtypes

import concourse.bass as bass
import concourse.bacc as bacc
import concourse.mybir as mybir
import concourse.tile as tile
from concourse import bass_utils, masks

F8np = ml_dtypes.float8_e4m3     # TRN FP8_EXP4-compatible (bias 7, max 240)
BFnp = ml_dtypes.bfloat16
F32 = mybir.dt.float32
BF16 = mybir.dt.bfloat16
F8 = mybir.dt.float8e4
DR = mybir.MatmulPerfMode.DoubleRow

N_CORES = 8
B, HW, C, D, P, Q = 8, 784, 1024, 2048, 128, 65536
QSH = Q // N_CORES          # 8192 queue rows per core
CT, DT = C // 128, D // 128  # 8, 16
NT = B * HW                 # 6272 total dense rows
RT = NT // 128              # 49 flat q-row tiles
TAU = 0.2
LAM = 0.5
ISC = 1.0 / TAU             # 5.0
AF = mybir.ActivationFunctionType
ALU = mybir.AluOpType

# 784 = 6*128 + 16 partition tiles (correspondence)
PT = [(i * 128, min(128, HW - i * 128)) for i in range(7)]
CH = [(0, 512), (512, HW - 512)]   # free-dim chunks of 784


def _patch_act_tables():
    """Force every activation we use onto the natural_log_exp_and_others
    table set so the kernel needs exactly one ACT_TABLE_LOAD."""
    import concourse.bacc as bacc_mod
    if getattr(bacc_mod, "_act_tables_patched", False):
        return
    from concourse import hw_specs
    orig = hw_specs.get_activation_tables
    ours = {AF.Exp, AF.Ln, AF.Relu, AF.Identity, AF.Copy, AF.Square}
    keep = "natural_log_exp_and_others"

    def patched(arch):
        tabs = orig(arch)
        assert keep in tabs and ours <= tabs[keep]
        return {name: (fns if name == keep else fns - ours)
                for name, fns in tabs.items()}

    bacc_mod.get_activation_tables = patched
    bacc_mod._act_tables_patched = True


def _build(do_compile=True):
    _patch_act_tables()
    nc = bacc.Bacc("TRN2", target_bir_lowering=False, debug=False,
                   num_devices=N_CORES)

    def inp(name, shape, dt):
        return nc.dram_tensor(name, list(shape), dt, kind="ExternalInput")

    env = {}
    env["xq_d"] = inp("xq", (128, CT * HW), F8)    # [c, ct*784+p]
    env["xk_d"] = inp("xk", (128, CT * HW), F8)
    env["wd1_d"] = inp("wd1", (DT, 128, C), F8)    # [dt, c, ct*128+d] x64
    env["wd1m_d"] = inp("wd1m", (DT, 128, C), F8)
    env["wd2_d"] = inp("wd2", (128, D), F8)        # [d, dt*128+p] x64
    env["wd2m_d"] = inp("wd2m", (128, D), F8)
    env["wg1_d"] = inp("wg1", (128, CT * D), F8)   # [c, ct*2048+d] x64
    env["wg1m_d"] = inp("wg1m", (128, CT * D), F8)
    env["wg2_d"] = inp("wg2", (128, D), F8)        # like wd2, x64
    env["wg2m_d"] = inp("wg2m", (128, D), F8)
    env["bd1_d"] = inp("bd1", (128, DT), F32)      # [r, dt] = bd1[dt*128+r]
    env["bd1m_d"] = inp("bd1m", (128, DT), F32)
    env["bd2_d"] = inp("bd2", (128, 1), F32)
    env["bd2m_d"] = inp("bd2m", (128, 1), F32)
    env["bg1r_d"] = inp("bg1r", (1, D), BF16)      # bg1 x4096 (bias row)
    env["bg1mr_d"] = inp("bg1mr", (1, D), BF16)
    env["bg2_d"] = inp("bg2", (128, 1), F32)
    env["bg2m_d"] = inp("bg2m", (128, 1), F32)
    env["queueT_d"] = inp("queueT", (128, QSH), F8)  # 8*queue[c0+j, ch]
    env["iota_d"] = inp("iota", (128, 1), F32)
    env["onesc_d"] = inp("onesc", (128, 1), F32)
    env["onesr_d"] = inp("onesr", (1, 128), F32)
    env["ones8_d"] = inp("ones8", (1, 8), BF16)

    env["outz_d"] = nc.dram_tensor("outz", [128, RT], F32,
                                   kind="ExternalOutput")
    env["outs_d"] = nc.dram_tensor("outs", [1, 16], F32,
                                   kind="ExternalOutput")

    with tile.TileContext(nc) as tc:
        with tc.tile_pool(name="dramp", bufs=1, space="DRAM") as dpool:
            env["ag_in"] = dpool.tile([128 * 785], F8, name="ag_in")
            env["ag_out"] = dpool.tile([N_CORES * 128 * 785], F8,
                                       name="ag_out", addr_space="Shared")
            with tc.tile_pool(name="cst", bufs=1) as cst:
                _body(nc, tc, env, cst)
    if do_compile:
        nc.compile()
    return nc


def _dense_branch(nc, tc, env, cst, br, pools, tiles):
    """One dense-head branch (q: br=0, k: br=1) -> normalized [128, HW]."""
    g = lambda k: env[k]
    sfx = "" if br == 0 else "m"
    x_sb = tiles["xq8" if br == 0 else "xk8"]
    w1_d = g("wd1" + sfx + "_d")
    w2_sb = tiles["wd2" + sfx]
    b1 = tiles["bd1" + sfx]
    b2 = tiles["bd2" + sfx]
    dst_bf = tiles["qdT_bf" if br == 0 else "kdT_bf"]
    w1p, hp, l2s, ps_h, ps_m = (pools["w1p"], pools["hp"], pools["l2s"],
                                pools["ps_h"], pools["ps_m"])

    xv = x_sb[:].rearrange("c (t p) -> c t p", t=CT)
    qd_ps = ps_m.tile([128, HW], F32, name=f"qd_ps{br}", tag="m")
    hq = None
    for dt in range(DT):
        w1t = w1p.tile([128, C], F8, name=f"w1t{br}")
        nc.sync.dma_start(w1t[:], w1_d[dt, :, :])
        h_ps = ps_h.tile([128, HW], F32, name="h_ps", tag="h")
        wv = w1t[:].rearrange("c (t d) -> c t d", t=CT)
        for kp in range(CT // 2):
            for (o, n) in CH:
                nc.tensor.matmul(
                    h_ps[:, o:o + n],
                    lhsT=wv[:, 2 * kp:2 * kp + 2, :],
                    rhs=xv[:, 2 * kp:2 * kp + 2, o:o + n],
                    start=(kp == 0), stop=(kp == CT // 2 - 1),
                    perf_mode=DR)
        if dt % 2 == 0:
            hq = hp.tile([128, 2 * HW], F8, name=f"hq{br}")
        nc.scalar.activation(hq[:, (dt % 2) * HW:(dt % 2 + 1) * HW],
                             h_ps[:], AF.Relu, bias=b1[:, dt:dt + 1],
                             scale=1.0 / 64.0)
        if br == 0:
            # stagger the k-branch / tail input DMAs on the scalar ring so
            # they don't compete with the q-critical loads
            if dt == 2:
                nc.scalar.dma_start(tiles["xk8"][:], g("xk_d")[:])
                # pooled sums for the k ghead: must be emitted after the
                # xk8 DMA (program order defines the dependency), runs on
                # the otherwise idle DVE during the q branch
                gsum = tiles["gsum1"]
                for ct in range(CT):
                    pscr = pools["pscr"].tile([128, HW], F8, name="pscr")
                    nc.vector.tensor_scalar(
                        pscr[:], tiles["xk8"][:, ct * HW:(ct + 1) * HW],
                        1.0, None, op0=ALU.mult, op1=ALU.add,
                        accum_out=gsum[:, ct:ct + 1])
            elif dt == 4:
                nc.scalar.dma_start(tiles["wg1"][:], g("wg1_d")[:])
                nc.scalar.dma_start(tiles["wg2"][:], g("wg2_d")[:])
            elif dt == 8:
                nc.scalar.dma_start(tiles["wd2m"][:], g("wd2m_d")[:])
                nc.scalar.dma_start(tiles["wg1m"][:], g("wg1m_d")[:])
            elif dt == 12:
                nc.scalar.dma_start(tiles["queueT8"][:], g("queueT_d")[:])
                nc.scalar.dma_start(tiles["wg2m"][:], g("wg2m_d")[:])
        if dt % 2 == 1:
            dp = dt // 2
            w2v = w2_sb[:].rearrange("c (t d) -> c t d", t=DT)
            hv = hq[:].rearrange("c (j p) -> c j p", j=2)
            for (o, n) in CH:
                nc.tensor.matmul(
                    qd_ps[:, o:o + n],
                    lhsT=w2v[:, 2 * dp:2 * dp + 2, :],
                    rhs=hv[:, :, o:o + n],
                    start=(dp == 0), stop=(dp == DT // 2 - 1),
                    perf_mode=DR)

    # bias + l2 normalize along channels (partition dim)
    qdT_f = l2s.tile([128, HW], F32, name=f"qdT_f{br}")
    nc.scalar.activation(qdT_f[:], qd_ps[:], AF.Identity, bias=b2[:],
                         scale=1.0 / 64.0)
    sq = l2s.tile([128, HW], BF16, name=f"sq{br}")
    nc.scalar.activation(sq[:], qdT_f[:], AF.Square)
    ssq_ps = ps_m.tile([1, HW], F32, name=f"ssq{br}", tag="m")
    for (o, n) in CH:
        nc.tensor.matmul(ssq_ps[:, o:o + n], lhsT=tiles["onescb"][:],
                         rhs=sq[:, o:o + n], start=True, stop=True)
    nrm = l2s.tile([1, HW], F32, name=f"nrm{br}")
    nc.vector.tensor_scalar_max(nrm[:], ssq_ps[:], 1e-12)
    nrm2 = l2s.tile([1, HW], F32, name=f"nrm2{br}")
    nc.scalar.activation(nrm2[:], nrm[:], AF.Ln)
    rn = l2s.tile([1, HW], F32, name=f"rn{br}")
    nc.scalar.activation(rn[:], nrm2[:], AF.Exp, scale=-0.5)
    rnb_ps = ps_m.tile([128, HW], F32, name=f"rnb{br}", tag="m")
    for (o, n) in CH:
        nc.tensor.matmul(rnb_ps[:, o:o + n], lhsT=tiles["onesr"][:],
                         rhs=rn[:, o:o + n], start=True, stop=True)
    nc.vector.tensor_mul(dst_bf[:], qdT_f[:], rnb_ps[:])
    return dst_bf


def _ghead_branch(nc, tc, env, cst, br, pools, tiles):
    """Global head for the core's own image (q: br=0, k: br=1)."""
    g = lambda k: env[k]
    sfx = "" if br == 0 else "m"
    x_sb = tiles["xq8" if br == 0 else "xk8"]
    w1_sb = tiles["wg1" + sfx]
    w2_sb = tiles["wg2" + sfx]
    b1r = tiles["bg1r" if br == 0 else "bg1mr"]
    b2 = tiles["bg2" + sfx]
    dst_bf = tiles["qgT_bf" if br == 0 else "kgT_bf"]
    gp, ps_m = pools["gp"], pools["ps_m"]
    ones1 = tiles["ones8"][0:1, 0:1]

    # pooled sums were computed up front on the DVE; scale to g*64 staged
    # at stride 16 for the DoubleRow stationary
    gsum = tiles[f"gsum{br}"]
    gqt8 = gp.tile([128, CT * 16], F8, name=f"gqt8{br}")
    gq_v = gqt8[:].rearrange("c (t s) -> c t s", s=16)
    nc.vector.tensor_scalar_mul(gq_v[:, :, 0:1], gsum[:], 64.0 / HW)

    # L1: h_g[1, 2048] = (g*64) @ (Wg1*64) / 4096 + bg1, in 512-chunks
    hgb = gp.tile([1, D], BF16, name=f"hgb{br}")
    w1v = w1_sb[:].rearrange("c (t d) -> c t d", t=CT)
    for ch in range(4):
        hg_ps = ps_m.tile([1, 512], F32, name=f"hg{br}", tag="m")
        for kp in range(CT // 2):
            nc.tensor.matmul(
                hg_ps[:], lhsT=gq_v[:, 2 * kp:2 * kp + 2, 0:1],
                rhs=w1v[:, 2 * kp:2 * kp + 2, ch * 512:(ch + 1) * 512],
                start=(kp == 0), stop=False, perf_mode=DR)
        nc.tensor.matmul(hg_ps[:], lhsT=ones1,
                         rhs=b1r[0:1, ch * 512:(ch + 1) * 512],
                         start=False, stop=True)
        nc.scalar.activation(hgb[0:1, ch * 512:(ch + 1) * 512], hg_ps[:],
                             AF.Relu, scale=1.0 / 4096.0)
    # transpose h_g -> [128, DT] via K=1 matmuls, then fp8 at stride 16
    hgt_ps = ps_m.tile([128, DT], F32, name=f"hgt{br}", tag="m")
    for dt in range(DT):
        nc.tensor.matmul(hgt_ps[:, dt:dt + 1],
                         lhsT=hgb[0:1, dt * 128:(dt + 1) * 128],
                         rhs=ones1, start=(dt == 0), stop=(dt == DT - 1))
    hgt8 = gp.tile([128, DT * 16], F8, name=f"hgt8{br}")
    hgt_v = hgt8[:].rearrange("c (t s) -> c t s", s=16)
    nc.scalar.activation(hgt_v[:, :, 0:1], hgt_ps[:], AF.Copy)
    # L2: q_g[128, 1]
    qg_ps = ps_m.tile([128, 1], F32, name=f"qg{br}", tag="m")
    w2v = w2_sb[:].rearrange("c (t d) -> c t d", t=DT)
    for dp in range(DT // 2):
        nc.tensor.matmul(qg_ps[:], lhsT=w2v[:, 2 * dp:2 * dp + 2, :],
                         rhs=hgt_v[:, 2 * dp:2 * dp + 2, 0:1],
                         start=(dp == 0), stop=(dp == DT // 2 - 1),
                         perf_mode=DR)
    qgT_f = gp.tile([128, 1], F32, name=f"qgT_f{br}")
    nc.scalar.activation(qgT_f[:], qg_ps[:], AF.Identity, bias=b2[:],
                         scale=1.0 / 64.0)
    sqg = gp.tile([128, 1], BF16, name=f"sqg{br}")
    nc.scalar.activation(sqg[:], qgT_f[:], AF.Square)
    ssg_ps = ps_m.tile([1, 1], F32, name=f"ssg{br}", tag="m")
    nc.tensor.matmul(ssg_ps[:], lhsT=tiles["onescb"][:], rhs=sqg[:],
                     start=True, stop=True)
    nrg = gp.tile([1, 1], F32, name=f"nrg{br}")
    nc.vector.tensor_scalar_max(nrg[:], ssg_ps[:], 1e-12)
    nrg2 = gp.tile([1, 1], F32, name=f"nrg2{br}")
    nc.scalar.activation(nrg2[:], nrg[:], AF.Ln)
    rng = gp.tile([1, 1], F32, name=f"rng{br}")
    nc.scalar.activation(rng[:], nrg2[:], AF.Exp, scale=-0.5)
    rngb_ps = ps_m.tile([128, 1], F32, name=f"rngb{br}", tag="m")
    nc.tensor.matmul(rngb_ps[:], lhsT=tiles["onesr"][:], rhs=rng[:],
                     start=True, stop=True)
    nc.vector.tensor_mul(dst_bf[:], qgT_f[:], rngb_ps[:])
    return dst_bf


def _body(nc, tc, env, cst):
    g = lambda k: env[k]
    tiles = {}

    # ---------------- inputs into SBUF ----------------
    tiles["xq8"] = cst.tile([128, CT * HW], F8, name="xq8")
    nc.sync.dma_start(tiles["xq8"][:], g("xq_d")[:])
    # scalar ring: only wd2 up front (needed at dt=1); the rest staggered
    tiles["wd2"] = cst.tile([128, D], F8, name="wd2")
    nc.scalar.dma_start(tiles["wd2"][:], g("wd2_d")[:])
    tiles["wg2"] = cst.tile([128, D], F8, name="wg2")
    tiles["wg1"] = cst.tile([128, CT * D], F8, name="wg1")
    # k-side tiles (DMAs staggered inside the q loop)
    tiles["xk8"] = cst.tile([128, CT * HW], F8, name="xk8")
    tiles["wd2m"] = cst.tile([128, D], F8, name="wd2m")
    tiles["wg1m"] = cst.tile([128, CT * D], F8, name="wg1m")
    tiles["wg2m"] = cst.tile([128, D], F8, name="wg2m")
    tiles["queueT8"] = cst.tile([128, QSH], F8, name="queueT8")
    # small consts on the gpsimd ring
    for nm, shp, dt in (("iota", (128, 1), F32), ("onesc", (128, 1), F32),
                        ("onesr", (1, 128), F32), ("ones8", (1, 8), BF16),
                        ("bd1", (128, DT), F32), ("bd1m", (128, DT), F32),
                        ("bd2", (128, 1), F32), ("bd2m", (128, 1), F32),
                        ("bg1r", (1, D), BF16), ("bg1mr", (1, D), BF16),
                        ("bg2", (128, 1), F32), ("bg2m", (128, 1), F32)):
        t = cst.tile(list(shp), dt, name=nm)
        nc.gpsimd.dma_start(t[:], g(nm + "_d")[:])
        tiles[nm] = t
    tiles["onescb"] = cst.tile([128, 1], BF16, name="onescb")
    nc.vector.tensor_copy(tiles["onescb"][:], tiles["onesc"][:])
    id_f = cst.tile([128, 128], F32, name="id_f")
    masks.make_identity(nc, id_f[:])
    id_b = cst.tile([128, 128], BF16, name="id_b")
    masks.make_identity(nc, id_b[:])

    # long-lived results
    for nm, shp, dt in (("qdT_bf", (128, HW), BF16),
                        ("kdT_bf", (128, HW), BF16),
                        ("qgT_bf", (128, 1), BF16),
                        ("kgT_bf", (128, 1), BF16),
                        ("qd8s", (128, 785), F8),
                        ("qall", (128, NT), F8),
                        ("qgall", (128, 8), F8),
                        ("matchT", (128, HW), BF16),
                        ("matchT8", (128, HW), F8),
                        ("zpart", (128, RT), F32),
                        ("fin", (1, 16), F32)):
        tiles[nm] = cst.tile(list(shp), dt, name=nm)
    nc.vector.memset(tiles["fin"][:], 0.0)

    pools = {}
    with tc.tile_pool(name="w1p", bufs=8) as pools["w1p"], \
         tc.tile_pool(name="hp", bufs=2) as pools["hp"], \
         tc.tile_pool(name="l2s", bufs=2) as pools["l2s"], \
         tc.tile_pool(name="gp", bufs=1) as pools["gp"], \
         tc.tile_pool(name="pscr", bufs=2) as pools["pscr"], \
         tc.tile_pool(name="ps_h", bufs=2, space="PSUM") as pools["ps_h"], \
         tc.tile_pool(name="ps_m", bufs=2, space="PSUM") as pools["ps_m"]:

        # pooled feature sums for the q ghead, up front on the idle DVE
        # (the k-side pooling is emitted right after the xk8 DMA below)
        tiles["gsum0"] = cst.tile([128, CT], F32, name="gsum0")
        tiles["gsum1"] = cst.tile([128, CT], F32, name="gsum1")
        for ct in range(CT):
            pscr = pools["pscr"].tile([128, HW], F8, name="pscr")
            nc.vector.tensor_scalar(
                pscr[:], tiles["xq8"][:, ct * HW:(ct + 1) * HW],
                1.0, None, op0=ALU.mult, op1=ALU.add,
                accum_out=tiles["gsum0"][:, ct:ct + 1])

        # ========== q branch + its global head, then the AllGather ==========
        _dense_branch(nc, tc, env, cst, 0, pools, tiles)
        nc.vector.tensor_scalar_mul(tiles["qd8s"][:, 0:HW],
                                    tiles["qdT_bf"][:], 8.0)
        _ghead_branch(nc, tc, env, cst, 0, pools, tiles)
        nc.vector.tensor_scalar_mul(tiles["qd8s"][:, HW:HW + 1],
                                    tiles["qgT_bf"][:], 8.0)
        ag_in, ag_out = g("ag_in"), g("ag_out")
        nc.gpsimd.dma_start(ag_in[:].rearrange("(c p) -> c p", c=128),
                            tiles["qd8s"][:])
        nc.gpsimd.collective_compute(
            "AllGather", ALU.bypass, replica_groups=[list(range(N_CORES))],
            ins=[ag_in.opt()], outs=[ag_out.opt()])

        # ========== k branch ==========
        _dense_branch(nc, tc, env, cst, 1, pools, tiles)

        # ========== correspondence (own image, bf16) ==========
        qdT_bf, kdT_bf = tiles["qdT_bf"], tiles["kdT_bf"]
        with tc.tile_pool(name="cor", bufs=1) as cor, \
             tc.tile_pool(name="cor2", bufs=2) as cor2:
            sim_sb = cor.tile([128, 7 * HW], BF16, name="sim_sb")
            for i, (po, pn) in enumerate(PT):
                s_ps = pools["ps_h"].tile([128, HW], F32, name="s_ps",
                                          tag="h")
                for (o, n) in CH:
                    nc.tensor.matmul(s_ps[0:pn, o:o + n],
                                     lhsT=qdT_bf[:, po:po + pn],
                                     rhs=kdT_bf[:, o:o + n],
                                     start=True, stop=True)
                nc.scalar.activation(sim_sb[0:pn, i * HW:i * HW + HW],
                                     s_ps[0:pn, :], AF.Copy)
            mx8 = cor.tile([128, 8], F32, name="mx8")
            ix8 = cor.tile([128, 8], mybir.dt.uint32, name="ix8")
            ixf = cor.tile([128, 7], F32, name="ixf")
            for i, (po, pn) in enumerate(PT):
                nc.vector.max(mx8[0:pn, :], sim_sb[0:pn, i * HW:i * HW + HW])
                nc.vector.max_index(ix8[0:pn, :], mx8[0:pn, :],
                                    sim_sb[0:pn, i * HW:i * HW + HW])
                nc.vector.tensor_copy(ixf[0:pn, i:i + 1], ix8[0:pn, 0:1])
            ir_sb = cor.tile([1, HW], F32, name="ir_sb")
            for i, (po, pn) in enumerate(PT):
                ir_ps = pools["ps_m"].tile([1, 128], F32, name="ir_ps",
                                           tag="m")
                nc.tensor.transpose(ir_ps[0:1, 0:pn], ixf[0:pn, i:i + 1],
                                    id_f[0:pn, 0:pn])
                nc.scalar.activation(ir_sb[0:1, po:po + pn],
                                     ir_ps[0:1, 0:pn], AF.Copy)
            ib_ps = pools["ps_m"].tile([128, HW], F32, name="ib_ps", tag="m")
            for (o, n) in CH:
                nc.tensor.matmul(ib_ps[:, o:o + n], lhsT=tiles["onesr"][:],
                                 rhs=ir_sb[:, o:o + n], start=True, stop=True)
            ib_sb = cor.tile([128, HW], F32, name="ib_sb")
            nc.scalar.activation(ib_sb[:], ib_ps[:], AF.Copy)
            # gather matched keys via one-hot matmuls; mt_ps stays resident
            # in ps_m while kt transposes rotate through ps_h
            mt_ps = pools["ps_m"].tile([128, HW], F32, name="mt_ps", tag="m")
            for i, (po, pn) in enumerate(PT):
                S = cor2.tile([128, HW], BF16, name="S")
                nc.vector.tensor_scalar(
                    S[0:pn, :], ib_sb[0:pn, :], tiles["iota"][0:pn, :],
                    float(po), op0=ALU.subtract, op1=ALU.is_equal)
                kt_ps = pools["ps_h"].tile([128, 128], BF16, name="kt_ps",
                                           tag="h")
                nc.tensor.transpose(kt_ps[0:pn, :], kdT_bf[:, po:po + pn],
                                    id_b[:, :])
                kt_sb = cor2.tile([128, 128], BF16, name="kt_sb")
                nc.scalar.activation(kt_sb[0:pn, :], kt_ps[0:pn, :], AF.Copy)
                for (o, n) in CH:
                    nc.tensor.matmul(mt_ps[:, o:o + n], lhsT=kt_sb[0:pn, :],
                                     rhs=S[0:pn, o:o + n],
                                     start=(i == 0), stop=(i == 6))
            nc.scalar.activation(tiles["matchT"][:], mt_ps[:], AF.Copy)
            nc.vector.tensor_scalar_mul(tiles["matchT8"][:], mt_ps[:], 8.0)

            # positives: diag = qd . matched (own rows), summed
            posm = cor.tile([128, HW], F32, name="posm")
            nc.vector.tensor_mul(posm[:], qdT_bf[:], tiles["matchT"][:])
            pos_ps = pools["ps_m"].tile([1, HW], F32, name="pos_ps", tag="m")
            for (o, n) in CH:
                nc.tensor.matmul(pos_ps[:, o:o + n], lhsT=tiles["onesc"][:],
                                 rhs=posm[:, o:o + n], start=True, stop=True)
            nc.vector.reduce_sum(tiles["fin"][0:1, 0:1], pos_ps[:],
                                 axis=mybir.AxisListType.X)

        # ========== k global head + lpos (off the critical path) ==========
        _ghead_branch(nc, tc, env, cst, 1, pools, tiles)
        lpm = pools["gp"].tile([128, 1], F32, name="lpm")
        nc.vector.tensor_mul(lpm[:], tiles["qgT_bf"][:], tiles["kgT_bf"][:])
        lp_ps = pools["ps_m"].tile([1, 1], F32, name="lp_ps", tag="m")
        nc.tensor.matmul(lp_ps[:], lhsT=tiles["onesc"][:], rhs=lpm[:],
                         start=True, stop=True)
        nc.vector.tensor_copy(tiles["fin"][0:1, 1:2], lp_ps[:])

        # ========== gathered q: dense logits first, then queue ==========
        agv = ag_out[:].rearrange("(r c p) -> c r p", r=N_CORES, c=128)
        nc.sync.dma_start(
            tiles["qgall"][:].rearrange("c (r p) -> c r p", p=1),
            agv[:, :, HW:HW + 1])
        nc.sync.dma_start(
            tiles["qall"][:].rearrange("c (r p) -> c r p", r=N_CORES),
            agv[:, :, 0:HW])

        with tc.tile_pool(name="escr", bufs=3) as escr:
            # dense logits, column shard: all 6272 q rows x own 784 keys;
            # per-row exp sums via DVE (keeps the ACT chain pure Exp).
            # The 8 queue-negative groups are interleaved into the
            # ACT-bound phase so their PE/ACT work fills the slack.
            qe_sb = cst.tile([128, 512], BF16, name="qe_sb")

            def qe_group(grp):
                qe_ps = pools["ps_m"].tile([128, 64], F32, name="qe_ps",
                                           tag="m")
                for j in range(8):
                    qt = grp * 8 + j
                    nc.tensor.matmul(
                        qe_ps[:, j * 8:(j + 1) * 8],
                        lhsT=tiles["queueT8"][:, qt * 128:(qt + 1) * 128],
                        rhs=tiles["qgall"][:], start=(j == 0), stop=(j == 7))
                nc.scalar.activation(qe_sb[:, grp * 64:(grp + 1) * 64],
                                     qe_ps[:], AF.Exp, scale=ISC / 64.0)

            for t in range(RT):
                lg_ps = pools["ps_h"].tile([128, HW], F32, name="lg_ps",
                                           tag="h")
                for (o, n) in CH:
                    nc.tensor.matmul(
                        lg_ps[:, o:o + n],
                        lhsT=tiles["qall"][:, t * 128:(t + 1) * 128],
                        rhs=tiles["matchT8"][:, o:o + n],
                        start=True, stop=True)
                es = escr.tile([128, HW], BF16, name="es")
                nc.scalar.activation(es[:], lg_ps[:], AF.Exp,
                                     scale=ISC / 64.0)
                nc.vector.reduce_sum(tiles["zpart"][:, t:t + 1], es[:],
                                     axis=mybir.AxisListType.X)
                if t >= 13 and (t - 13) % 5 == 0 and (t - 13) // 5 < 8:
                    qe_group((t - 13) // 5)
            qs_ps = pools["ps_m"].tile([1, 512], F32, name="qs_ps", tag="m")
            nc.tensor.matmul(qs_ps[:], lhsT=tiles["onescb"][:], rhs=qe_sb[:],
                             start=True, stop=True)
            nc.vector.reduce_sum(tiles["fin"][0:1, 2:10],
                                 qs_ps[:].rearrange("p (t i) -> p i t", i=8),
                                 axis=mybir.AxisListType.X)

        nc.sync.dma_start(g("outz_d")[:], tiles["zpart"][:])
        nc.sync.dma_start(g("outs_d")[:], tiles["fin"][:])


def _prep_inputs(inputs):
    fq = np.asarray(inputs["feat_q"], np.float32).reshape(B, HW, C)
    fk = np.asarray(inputs["feat_k"], np.float32).reshape(B, HW, C)

    def xT8(x):  # (784, 1024) -> (128, 8*784) f8 with [c, ct*784+p]
        return np.ascontiguousarray(
            x.reshape(HW, CT, 128).transpose(2, 1, 0).reshape(128, CT * HW)
        ).astype(F8np)

    def w1tile(w):  # (1024, 2048) -> (16, 128, 1024) f8 x64
        return np.ascontiguousarray(
            (w * 64.0).reshape(CT, 128, DT, 128).transpose(2, 1, 0, 3)
            .reshape(DT, 128, C)).astype(F8np)

    def w2tile(w):  # (2048, 128) -> (128, 2048) f8 x64
        return np.ascontiguousarray(
            (w * 64.0).reshape(DT, 128, 128).transpose(1, 0, 2)
            .reshape(128, D)).astype(F8np)

    def wg1tile(w):  # (1024, 2048) -> (128, 8*2048) f8 x64
        return np.ascontiguousarray(
            (w * 64.0).reshape(CT, 128, D).transpose(1, 0, 2)
            .reshape(128, CT * D)).astype(F8np)

    shared = {
        "wd1": w1tile(inputs["Wd1"]), "wd1m": w1tile(inputs["mWd1"]),
        "wd2": w2tile(inputs["Wd2"]), "wd2m": w2tile(inputs["mWd2"]),
        "wg1": wg1tile(inputs["Wg1"]), "wg1m": wg1tile(inputs["mWg1"]),
        "wg2": w2tile(inputs["Wg2"]), "wg2m": w2tile(inputs["mWg2"]),
        "bd1": np.ascontiguousarray(
            np.asarray(inputs["bd1"], np.float32).reshape(DT, 128).T),
        "bd1m": np.ascontiguousarray(
            np.asarray(inputs["mbd1"], np.float32).reshape(DT, 128).T),
        "bd2": np.asarray(inputs["bd2"], np.float32).reshape(128, 1),
        "bd2m": np.asarray(inputs["mbd2"], np.float32).reshape(128, 1),
        "bg1r": (np.asarray(inputs["bg1"], np.float32) * 4096.0
                 ).reshape(1, D).astype(BFnp),
        "bg1mr": (np.asarray(inputs["mbg1"], np.float32) * 4096.0
                  ).reshape(1, D).astype(BFnp),
        "bg2": np.asarray(inputs["bg2"], np.float32).reshape(128, 1),
        "bg2m": np.asarray(inputs["mbg2"], np.float32).reshape(128, 1),
        "iota": np.arange(128, dtype=np.float32).reshape(128, 1),
        "onesc": np.ones((128, 1), np.float32),
        "onesr": np.ones((1, 128), np.float32),
        "ones8": np.ones((1, 8), np.float32).astype(BFnp),
    }
    queue = np.asarray(inputs["queue"], np.float32)
    in_maps = []
    for c in range(N_CORES):
        m = dict(shared)
        m["xq"] = xT8(fq[c])
        m["xk"] = xT8(fk[c])
        m["queueT"] = np.ascontiguousarray(
            (queue[c * QSH:(c + 1) * QSH] * 8.0).T).astype(F8np)
        in_maps.append(m)
    return in_maps


_NC = None


def _get_nc():
    global _NC
    if _NC is None:
        _NC = _build()
    return _NC


def _host_combine(outz, outs):
    """outz: [8][128, 49] z-partials; outs: [8][1, 16] scalars.

    outs slots: [0] sum(qd.matched) over own rows, [1] own-image lpos,
    [2:10] partial sum(exp(l_neg/tau)) per image over the core's queue
    shard.  Dense z row r=t*128+p lives at outz[:, p, t].
    """
    outz = np.asarray(outz, np.float64)   # [8, 128, 49]
    outs = np.asarray(outs, np.float64)   # [8, 16]
    z = outz.sum(axis=0)                  # [128, 49]
    zrows = z.T.reshape(-1)               # row r = t*128+p
    pos_total = outs[:, 0].sum()
    l_d = (np.log(zrows).sum() - ISC * pos_total) / NT
    zq = outs[:, 2:10].sum(axis=0)        # [8]
    lpos = outs[np.arange(8), 1]          # core c owns image c
    lse = np.log(zq + np.exp(ISC * lpos))
    l_g = np.mean(lse - ISC * lpos)
    return np.float32((1.0 - LAM) * l_g + LAM * l_d).reshape(())


def kernel(**inputs) -> np.ndarray:
    nc = _get_nc()
    in_maps = _prep_inputs(inputs)
    res = bass_utils.run_bass_kernel_spmd(nc, in_maps,
                                          core_ids=list(range(N_CORES)))
    outz = np.stack([res.results[c]["outz"] for c in range(N_CORES)])
    outs = np.stack([res.results[c]["outs"].reshape(16)
                     for c in range(N_CORES)])
    return _host_combine(outz, outs)


# revision 31
# speedup vs baseline: 1.6164x; 1.0239x over previous
"""DenseCL loss kernel for 8 TRN2 NeuronCores (v2: fp8 DoubleRow + column-
sharded dense-InfoNCE logits).

Sharding: core c owns image c (dense head + correspondence + matched keys),
queue rows [c*8192, (c+1)*8192), and the COLUMN shard of the flat dense
logits: core c computes partial exp-sums over its own 784 matched-key
columns for ALL 6272 q rows; the host sums the per-core z partials.  The
only critical-path collective is a single early AllGather of the fp8
q_d (+ q_g) launched right after the q branch, hidden under the k branch.

Dense/global head matmuls run in fp8e4 with DoubleRow (2 contraction rows
per PE cell); weights are pre-scaled x64 on the host, the 1/64 folds into
the activation scale.  End-to-end fp8 rel-err vs the fp32 reference is
~5e-4 (validated in numpy), far under the 2e-2 gate.
"""
import sys

if "/opt/trn_rl_repo" not in sys.path:
    sys.path.insert(0, "/opt/trn_rl_repo")

import numpy as np
import ml_dtypes

import concourse.bass as bass
import concourse.bacc as bacc
import concourse.mybir as mybir
import concourse.tile as tile
from concourse import bass_utils, masks

F8np = ml_dtypes.float8_e4m3     # TRN FP8_EXP4-compatible (bias 7, max 240)
BFnp = ml_dtypes.bfloat16
F32 = mybir.dt.float32
BF16 = mybir.dt.bfloat16
F8 = mybir.dt.float8e4
DR = mybir.MatmulPerfMode.DoubleRow

N_CORES = 8
B, HW, C, D, P, Q = 8, 784, 1024, 2048, 128, 65536
QSH = Q // N_CORES          # 8192 queue rows per core
CT, DT = C // 128, D // 128  # 8, 16
NT = B * HW                 # 6272 total dense rows
RT = NT // 128              # 49 flat q-row tiles
TAU = 0.2
LAM = 0.5
ISC = 1.0 / TAU             # 5.0
AF = mybir.ActivationFunctionType
ALU = mybir.AluOpType

# 784 = 6*128 + 16 partition tiles (correspondence)
PT = [(i * 128, min(128, HW - i * 128)) for i in range(7)]
CH = [(0, 512), (512, HW - 512)]   # free-dim chunks of 784


def _patch_act_tables():
    """Force every activation we use onto the natural_log_exp_and_others
    table set so the kernel needs exactly one ACT_TABLE_LOAD."""
    import concourse.bacc as bacc_mod
    if getattr(bacc_mod, "_act_tables_patched", False):
        return
    from concourse import hw_specs
    orig = hw_specs.get_activation_tables
    ours = {AF.Exp, AF.Ln, AF.Relu, AF.Identity, AF.Copy, AF.Square}
    keep = "natural_log_exp_and_others"

    def patched(arch):
        tabs = orig(arch)
        assert keep in tabs and ours <= tabs[keep]
        return {name: (fns if name == keep else fns - ours)
                for name, fns in tabs.items()}

    bacc_mod.get_activation_tables = patched
    bacc_mod._act_tables_patched = True


def _build(do_compile=True):
    _patch_act_tables()
    nc = bacc.Bacc("TRN2", target_bir_lowering=False, debug=False,
                   num_devices=N_CORES)

    def inp(name, shape, dt):
        return nc.dram_tensor(name, list(shape), dt, kind="ExternalInput")

    env = {}
    env["xq_d"] = inp("xq", (128, CT * HW), F8)    # [c, ct*784+p]
    env["xk_d"] = inp("xk", (128, CT * HW), F8)
    env["wd1_d"] = inp("wd1", (DT, 128, C), F8)    # [dt, c, ct*128+d] x64
    env["wd1m_d"] = inp("wd1m", (DT, 128, C), F8)
    env["wd2_d"] = inp("wd2", (128, D), F8)        # [d, dt*128+p] x64
    env["wd2m_d"] = inp("wd2m", (128, D), F8)
    env["wg1_d"] = inp("wg1", (128, CT * D), F8)   # [c, ct*2048+d] x64
    env["wg1m_d"] = inp("wg1m", (128, CT * D), F8)
    env["wg2_d"] = inp("wg2", (128, D), F8)        # like wd2, x64
    env["wg2m_d"] = inp("wg2m", (128, D), F8)
    env["bd1_d"] = inp("bd1", (128, DT), F32)      # [r, dt] = bd1[dt*128+r]
    env["bd1m_d"] = inp("bd1m", (128, DT), F32)
    env["bd2_d"] = inp("bd2", (128, 1), F32)
    env["bd2m_d"] = inp("bd2m", (128, 1), F32)
    env["bg1r_d"] = inp("bg1r", (1, D), BF16)      # bg1 x4096 (bias row)
    env["bg1mr_d"] = inp("bg1mr", (1, D), BF16)
    env["bg2_d"] = inp("bg2", (128, 1), F32)
    env["bg2m_d"] = inp("bg2m", (128, 1), F32)
    env["queueT_d"] = inp("queueT", (128, QSH), F8)  # 8*queue[c0+j, ch]
    env["iota_d"] = inp("iota", (128, 1), F32)
    env["onesc_d"] = inp("onesc", (128, 1), F32)
    env["onesr_d"] = inp("onesr", (1, 128), F32)
    env["ones8_d"] = inp("ones8", (1, 8), BF16)

    env["outz_d"] = nc.dram_tensor("outz", [128, RT], F32,
                                   kind="ExternalOutput")
    env["outs_d"] = nc.dram_tensor("outs", [1, 16], F32,
                                   kind="ExternalOutput")

    with tile.TileContext(nc) as tc:
        with tc.tile_pool(name="dramp", bufs=1, space="DRAM") as dpool:
            env["ag_in"] = dpool.tile([128 * 785], F8, name="ag_in")
            env["ag_out"] = dpool.tile([N_CORES * 128 * 785], F8,
                                       name="ag_out", addr_space="Shared")
            with tc.tile_pool(name="cst", bufs=1) as cst:
                _body(nc, tc, env, cst)
    if do_compile:
        nc.compile()
    return nc


def _dense_branch(nc, tc, env, cst, br, pools, tiles):
    """One dense-head branch (q: br=0, k: br=1) -> normalized [128, HW]."""
    g = lambda k: env[k]
    sfx = "" if br == 0 else "m"
    x_sb = tiles["xq8" if br == 0 else "xk8"]
    w1_d = g("wd1" + sfx + "_d")
    w2_sb = tiles["wd2" + sfx]
    b1 = tiles["bd1" + sfx]
    b2 = tiles["bd2" + sfx]
    dst_bf = tiles["qdT_bf" if br == 0 else "kdT_bf"]
    w1p, hp, l2s, ps_h, ps_m = (pools["w1p"], pools["hp"], pools["l2s"],
                                pools["ps_h"], pools["ps_m"])

    xv = x_sb[:].rearrange("c (t p) -> c t p", t=CT)
    qd_ps = ps_m.tile([128, HW], F32, name=f"qd_ps{br}", tag="m")
    hq = None
    hq_prev = None

    def l2_pair(hsrc, dp):
        w2v = w2_sb[:].rearrange("c (t d) -> c t d", t=DT)
        hv = hsrc[:].rearrange("c (j p) -> c j p", j=2)
        for (o, n) in CH:
            nc.tensor.matmul(
                qd_ps[:, o:o + n],
                lhsT=w2v[:, 2 * dp:2 * dp + 2, :],
                rhs=hv[:, :, o:o + n],
                start=(dp == 0), stop=(dp == DT // 2 - 1),
                perf_mode=DR)

    for dt in range(DT):
        w1t = w1p.tile([128, C], F8, name=f"w1t{br}")
        nc.sync.dma_start(w1t[:], w1_d[dt, :, :])
        h_ps = ps_h.tile([128, HW], F32, name="h_ps", tag="h")
        wv = w1t[:].rearrange("c (t d) -> c t d", t=CT)
        for kp in range(CT // 2):
            for (o, n) in CH:
                nc.tensor.matmul(
                    h_ps[:, o:o + n],
                    lhsT=wv[:, 2 * kp:2 * kp + 2, :],
                    rhs=xv[:, 2 * kp:2 * kp + 2, o:o + n],
                    start=(kp == 0), stop=(kp == CT // 2 - 1),
                    perf_mode=DR)
        # L2 for the pair two dts back: by now its relus have finished, so
        # the PE never stalls waiting on the ACT chain
        if dt % 2 == 0 and dt >= 2:
            l2_pair(hq, dt // 2 - 1)
        if dt % 2 == 0:
            hq = hp.tile([128, 2 * HW], F8, name=f"hq{br}")
        nc.scalar.activation(hq[:, (dt % 2) * HW:(dt % 2 + 1) * HW],
                             h_ps[:], AF.Relu, bias=b1[:, dt:dt + 1],
                             scale=1.0 / 64.0)
        if br == 0:
            # stagger the k-branch / tail input DMAs on the scalar ring so
            # they don't compete with the q-critical loads
            if dt == 2:
                nc.scalar.dma_start(tiles["xk8"][:], g("xk_d")[:])
                # pooled sums for the k ghead: must be emitted after the
                # xk8 DMA (program order defines the dependency), runs on
                # the otherwise idle DVE during the q branch
                gsum = tiles["gsum1"]
                for ct in range(CT):
                    pscr = pools["pscr"].tile([128, HW], F8, name="pscr")
                    nc.vector.tensor_scalar(
                        pscr[:], tiles["xk8"][:, ct * HW:(ct + 1) * HW],
                        1.0, None, op0=ALU.mult, op1=ALU.add,
                        accum_out=gsum[:, ct:ct + 1])
                    if ct == 1:
                        for nm in ("wd2m", "wg1m", "wg2m", "queueT8"):
                            nc.vector.memset(tiles[nm][:, 0:1], 0.0)
            elif dt == 4:
                nc.scalar.dma_start(tiles["wg1"][:], g("wg1_d")[:])
                nc.scalar.dma_start(tiles["wg2"][:], g("wg2_d")[:])
            elif dt == 8:
                nc.scalar.dma_start(tiles["wd2m"][:], g("wd2m_d")[:])
                nc.scalar.dma_start(tiles["wg1m"][:], g("wg1m_d")[:])
            elif dt == 12:
                nc.scalar.dma_start(tiles["queueT8"][:], g("queueT_d")[:])
                nc.scalar.dma_start(tiles["wg2m"][:], g("wg2m_d")[:])

    l2_pair(hq, DT // 2 - 1)

    # bias + l2 normalize along channels (partition dim)
    qdT_f = l2s.tile([128, HW], F32, name=f"qdT_f{br}")
    nc.scalar.activation(qdT_f[:], qd_ps[:], AF.Identity, bias=b2[:],
                         scale=1.0 / 64.0)
    sq = l2s.tile([128, HW], BF16, name=f"sq{br}")
    nc.scalar.activation(sq[:], qdT_f[:], AF.Square)
    ssq_ps = ps_m.tile([1, HW], F32, name=f"ssq{br}", tag="m")
    for (o, n) in CH:
        nc.tensor.matmul(ssq_ps[:, o:o + n], lhsT=tiles["onescb"][:],
                         rhs=sq[:, o:o + n], start=True, stop=True)
    nrm = l2s.tile([1, HW], F32, name=f"nrm{br}")
    nc.vector.tensor_scalar_max(nrm[:], ssq_ps[:], 1e-12)
    nrm2 = l2s.tile([1, HW], F32, name=f"nrm2{br}")
    nc.scalar.activation(nrm2[:], nrm[:], AF.Ln)
    rn = l2s.tile([1, HW], F32, name=f"rn{br}")
    nc.scalar.activation(rn[:], nrm2[:], AF.Exp, scale=-0.5)
    rnb_ps = ps_m.tile([128, HW], F32, name=f"rnb{br}", tag="m")
    for (o, n) in CH:
        nc.tensor.matmul(rnb_ps[:, o:o + n], lhsT=tiles["onesr"][:],
                         rhs=rn[:, o:o + n], start=True, stop=True)
    nc.vector.tensor_mul(dst_bf[:], qdT_f[:], rnb_ps[:])
    return dst_bf


def _ghead_branch(nc, tc, env, cst, br, pools, tiles):
    """Global head for the core's own image (q: br=0, k: br=1)."""
    g = lambda k: env[k]
    sfx = "" if br == 0 else "m"
    x_sb = tiles["xq8" if br == 0 else "xk8"]
    w1_sb = tiles["wg1" + sfx]
    w2_sb = tiles["wg2" + sfx]
    b1r = tiles["bg1r" if br == 0 else "bg1mr"]
    b2 = tiles["bg2" + sfx]
    dst_bf = tiles["qgT_bf" if br == 0 else "kgT_bf"]
    gp, ps_m = pools["gp"], pools["ps_m"]
    ones1 = tiles["ones8"][0:1, 0:1]

    # pooled sums were computed up front on the DVE; scale to g*64 staged
    # at stride 16 for the DoubleRow stationary
    gsum = tiles[f"gsum{br}"]
    gqt8 = gp.tile([128, CT * 16], F8, name=f"gqt8{br}")
    gq_v = gqt8[:].rearrange("c (t s) -> c t s", s=16)
    nc.vector.tensor_scalar_mul(gq_v[:, :, 0:1], gsum[:], 64.0 / HW)

    # L1: h_g[1, 2048] = (g*64) @ (Wg1*64) / 4096 + bg1, in 512-chunks
    hgb = gp.tile([1, D], BF16, name=f"hgb{br}")
    w1v = w1_sb[:].rearrange("c (t d) -> c t d", t=CT)
    for ch in range(4):
        hg_ps = ps_m.tile([1, 512], F32, name=f"hg{br}", tag="m")
        for kp in range(CT // 2):
            nc.tensor.matmul(
                hg_ps[:], lhsT=gq_v[:, 2 * kp:2 * kp + 2, 0:1],
                rhs=w1v[:, 2 * kp:2 * kp + 2, ch * 512:(ch + 1) * 512],
                start=(kp == 0), stop=False, perf_mode=DR)
        nc.tensor.matmul(hg_ps[:], lhsT=ones1,
                         rhs=b1r[0:1, ch * 512:(ch + 1) * 512],
                         start=False, stop=True)
        nc.scalar.activation(hgb[0:1, ch * 512:(ch + 1) * 512], hg_ps[:],
                             AF.Relu, scale=1.0 / 4096.0)
    # transpose h_g -> [128, DT] via K=1 matmuls, then fp8 at stride 16
    hgt_ps = ps_m.tile([128, DT], F32, name=f"hgt{br}", tag="m")
    for dt in range(DT):
        nc.tensor.matmul(hgt_ps[:, dt:dt + 1],
                         lhsT=hgb[0:1, dt * 128:(dt + 1) * 128],
                         rhs=ones1, start=(dt == 0), stop=(dt == DT - 1))
    hgt8 = gp.tile([128, DT * 16], F8, name=f"hgt8{br}")
    hgt_v = hgt8[:].rearrange("c (t s) -> c t s", s=16)
    nc.scalar.activation(hgt_v[:, :, 0:1], hgt_ps[:], AF.Copy)
    # L2: q_g[128, 1]
    qg_ps = ps_m.tile([128, 1], F32, name=f"qg{br}", tag="m")
    w2v = w2_sb[:].rearrange("c (t d) -> c t d", t=DT)
    for dp in range(DT // 2):
        nc.tensor.matmul(qg_ps[:], lhsT=w2v[:, 2 * dp:2 * dp + 2, :],
                         rhs=hgt_v[:, 2 * dp:2 * dp + 2, 0:1],
                         start=(dp == 0), stop=(dp == DT // 2 - 1),
                         perf_mode=DR)
    qgT_f = gp.tile([128, 1], F32, name=f"qgT_f{br}")
    nc.scalar.activation(qgT_f[:], qg_ps[:], AF.Identity, bias=b2[:],
                         scale=1.0 / 64.0)
    sqg = gp.tile([128, 1], BF16, name=f"sqg{br}")
    nc.scalar.activation(sqg[:], qgT_f[:], AF.Square)
    ssg_ps = ps_m.tile([1, 1], F32, name=f"ssg{br}", tag="m")
    nc.tensor.matmul(ssg_ps[:], lhsT=tiles["onescb"][:], rhs=sqg[:],
                     start=True, stop=True)
    nrg = gp.tile([1, 1], F32, name=f"nrg{br}")
    nc.vector.tensor_scalar_max(nrg[:], ssg_ps[:], 1e-12)
    nrg2 = gp.tile([1, 1], F32, name=f"nrg2{br}")
    nc.scalar.activation(nrg2[:], nrg[:], AF.Ln)
    rng = gp.tile([1, 1], F32, name=f"rng{br}")
    nc.scalar.activation(rng[:], nrg2[:], AF.Exp, scale=-0.5)
    rngb_ps = ps_m.tile([128, 1], F32, name=f"rngb{br}", tag="m")
    nc.tensor.matmul(rngb_ps[:], lhsT=tiles["onesr"][:], rhs=rng[:],
                     start=True, stop=True)
    nc.vector.tensor_mul(dst_bf[:], qgT_f[:], rngb_ps[:])
    return dst_bf


def _body(nc, tc, env, cst):
    g = lambda k: env[k]
    tiles = {}

    # ---------------- inputs into SBUF ----------------
    tiles["xq8"] = cst.tile([128, CT * HW], F8, name="xq8")
    nc.sync.dma_start(tiles["xq8"][:], g("xq_d")[:])
    # scalar ring: only wd2 up front (needed at dt=1); the rest staggered
    tiles["wd2"] = cst.tile([128, D], F8, name="wd2")
    nc.scalar.dma_start(tiles["wd2"][:], g("wd2_d")[:])
    tiles["wg2"] = cst.tile([128, D], F8, name="wg2")
    tiles["wg1"] = cst.tile([128, CT * D], F8, name="wg1")
    # k-side tiles (DMAs staggered inside the q loop)
    tiles["xk8"] = cst.tile([128, CT * HW], F8, name="xk8")
    tiles["wd2m"] = cst.tile([128, D], F8, name="wd2m")
    tiles["wg1m"] = cst.tile([128, CT * D], F8, name="wg1m")
    tiles["wg2m"] = cst.tile([128, D], F8, name="wg2m")
    tiles["queueT8"] = cst.tile([128, QSH], F8, name="queueT8")
    # small consts on the gpsimd ring
    for nm, shp, dt in (("iota", (128, 1), F32), ("onesc", (128, 1), F32),
                        ("onesr", (1, 128), F32), ("ones8", (1, 8), BF16),
                        ("bd1", (128, DT), F32), ("bd1m", (128, DT), F32),
                        ("bd2", (128, 1), F32), ("bd2m", (128, 1), F32),
                        ("bg1r", (1, D), BF16), ("bg1mr", (1, D), BF16),
                        ("bg2", (128, 1), F32), ("bg2m", (128, 1), F32)):
        t = cst.tile(list(shp), dt, name=nm)
        nc.gpsimd.dma_start(t[:], g(nm + "_d")[:])
        tiles[nm] = t
    tiles["onescb"] = cst.tile([128, 1], BF16, name="onescb")
    nc.vector.tensor_copy(tiles["onescb"][:], tiles["onesc"][:])
    id_f = cst.tile([128, 128], F32, name="id_f")
    masks.make_identity(nc, id_f[:])
    id_b = cst.tile([128, 128], BF16, name="id_b")
    masks.make_identity(nc, id_b[:])

    # long-lived results
    for nm, shp, dt in (("qdT_bf", (128, HW), BF16),
                        ("kdT_bf", (128, HW), BF16),
                        ("qgT_bf", (128, 1), BF16),
                        ("kgT_bf", (128, 1), BF16),
                        ("qd8s", (128, 785), F8),
                        ("qall", (128, NT), F8),
                        ("qgall", (128, 8), F8),
                        ("matchT", (128, HW), BF16),
                        ("matchT8", (128, HW), F8),
                        ("zpart", (128, RT), F32),
                        ("fin", (1, 16), F32)):
        tiles[nm] = cst.tile(list(shp), dt, name=nm)
    nc.vector.memset(tiles["fin"][:], 0.0)

    pools = {}
    with tc.tile_pool(name="w1p", bufs=8) as pools["w1p"], \
         tc.tile_pool(name="hp", bufs=2) as pools["hp"], \
         tc.tile_pool(name="l2s", bufs=2) as pools["l2s"], \
         tc.tile_pool(name="gp", bufs=1) as pools["gp"], \
         tc.tile_pool(name="pscr", bufs=2) as pools["pscr"], \
         tc.tile_pool(name="ps_h", bufs=2, space="PSUM") as pools["ps_h"], \
         tc.tile_pool(name="ps_m", bufs=2, space="PSUM") as pools["ps_m"]:

        # pooled feature sums for the q ghead, up front on the idle DVE
        # (the k-side pooling is emitted right after the xk8 DMA below)
        tiles["gsum0"] = cst.tile([128, CT], F32, name="gsum0")
        tiles["gsum1"] = cst.tile([128, CT], F32, name="gsum1")
        for ct in range(CT):
            pscr = pools["pscr"].tile([128, HW], F8, name="pscr")
            nc.vector.tensor_scalar(
                pscr[:], tiles["xq8"][:, ct * HW:(ct + 1) * HW],
                1.0, None, op0=ALU.mult, op1=ALU.add,
                accum_out=tiles["gsum0"][:, ct:ct + 1])
            if ct == 1:
                # gate the next wave of input DMAs behind this point of
                # the DVE stream: a dummy first-writer makes the (otherwise
                # dependency-free) loads wait, so they cannot steal HBM
                # bandwidth from the critical xq/wd1 stream at t=0
                for nm in ("xk8", "wg1", "wg2"):
                    nc.vector.memset(tiles[nm][:, 0:1], 0.0)

        # ========== q branch + its global head, then the AllGather ==========
        _dense_branch(nc, tc, env, cst, 0, pools, tiles)
        nc.vector.tensor_scalar_mul(tiles["qd8s"][:, 0:HW],
                                    tiles["qdT_bf"][:], 8.0)
        _ghead_branch(nc, tc, env, cst, 0, pools, tiles)
        nc.vector.tensor_scalar_mul(tiles["qd8s"][:, HW:HW + 1],
                                    tiles["qgT_bf"][:], 8.0)
        ag_in, ag_out = g("ag_in"), g("ag_out")
        nc.gpsimd.dma_start(ag_in[:].rearrange("(c p) -> c p", c=128),
                            tiles["qd8s"][:])
        nc.gpsimd.collective_compute(
            "AllGather", ALU.bypass, replica_groups=[list(range(N_CORES))],
            ins=[ag_in.opt()], outs=[ag_out.opt()])

        # ========== k branch ==========
        _dense_branch(nc, tc, env, cst, 1, pools, tiles)

        # AG-output loads (wait on the collective, nothing else on sync)
        agv = ag_out[:].rearrange("(r c p) -> c r p", r=N_CORES, c=128)
        nc.sync.dma_start(
            tiles["qgall"][:].rearrange("c (r p) -> c r p", p=1),
            agv[:, :, HW:HW + 1])
        nc.sync.dma_start(
            tiles["qall"][:].rearrange("c (r p) -> c r p", r=N_CORES),
            agv[:, :, 0:HW])

        # ========== correspondence (own image, bf16) ==========
        qdT_bf, kdT_bf = tiles["qdT_bf"], tiles["kdT_bf"]
        with tc.tile_pool(name="cor", bufs=1) as cor, \
             tc.tile_pool(name="cor2", bufs=2) as cor2:
            sim_sb = cor.tile([128, 7 * HW], BF16, name="sim_sb")
            for i, (po, pn) in enumerate(PT):
                s_ps = pools["ps_h"].tile([128, HW], F32, name="s_ps",
                                          tag="h")
                for (o, n) in CH:
                    nc.tensor.matmul(s_ps[0:pn, o:o + n],
                                     lhsT=qdT_bf[:, po:po + pn],
                                     rhs=kdT_bf[:, o:o + n],
                                     start=True, stop=True)
                nc.scalar.activation(sim_sb[0:pn, i * HW:i * HW + HW],
                                     s_ps[0:pn, :], AF.Copy)
            mx8 = cor.tile([128, 8], F32, name="mx8")
            ix8 = cor.tile([128, 8], mybir.dt.uint32, name="ix8")
            ixf = cor.tile([128, 7], F32, name="ixf")
            for i, (po, pn) in enumerate(PT):
                nc.vector.max(mx8[0:pn, :], sim_sb[0:pn, i * HW:i * HW + HW])
                nc.vector.max_index(ix8[0:pn, :], mx8[0:pn, :],
                                    sim_sb[0:pn, i * HW:i * HW + HW])
                nc.vector.tensor_copy(ixf[0:pn, i:i + 1], ix8[0:pn, 0:1])

            # k global head + lpos here: its PE work fills the DVE-argmax
            # window, keeping the PE busy (and the HAM clock warm)
            _ghead_branch(nc, tc, env, cst, 1, pools, tiles)
            lpm = pools["gp"].tile([128, 1], F32, name="lpm")
            nc.vector.tensor_mul(lpm[:], tiles["qgT_bf"][:],
                                 tiles["kgT_bf"][:])
            lp_ps = pools["ps_m"].tile([1, 1], F32, name="lp_ps", tag="m")
            nc.tensor.matmul(lp_ps[:], lhsT=tiles["onesc"][:], rhs=lpm[:],
                             start=True, stop=True)
            nc.vector.tensor_copy(tiles["fin"][0:1, 1:2], lp_ps[:])

            ir_sb = cor.tile([1, HW], F32, name="ir_sb")
            for i, (po, pn) in enumerate(PT):
                ir_ps = pools["ps_m"].tile([1, 128], F32, name="ir_ps",
                                           tag="m")
                nc.tensor.transpose(ir_ps[0:1, 0:pn], ixf[0:pn, i:i + 1],
                                    id_f[0:pn, 0:pn])
                nc.scalar.activation(ir_sb[0:1, po:po + pn],
                                     ir_ps[0:1, 0:pn], AF.Copy)
            ib_ps = pools["ps_m"].tile([128, HW], F32, name="ib_ps", tag="m")
            for (o, n) in CH:
                nc.tensor.matmul(ib_ps[:, o:o + n], lhsT=tiles["onesr"][:],
                                 rhs=ir_sb[:, o:o + n], start=True, stop=True)
            ib_sb = cor.tile([128, HW], F32, name="ib_sb")
            nc.scalar.activation(ib_sb[:], ib_ps[:], AF.Copy)
            # gather matched keys via one-hot matmuls; mt_ps stays resident
            # in ps_m while kt transposes rotate through ps_h
            mt_ps = pools["ps_m"].tile([128, HW], F32, name="mt_ps", tag="m")
            for i, (po, pn) in enumerate(PT):
                S = cor2.tile([128, HW], BF16, name="S")
                nc.vector.tensor_scalar(
                    S[0:pn, :], ib_sb[0:pn, :], tiles["iota"][0:pn, :],
                    float(po), op0=ALU.subtract, op1=ALU.is_equal)
                kt_ps = pools["ps_h"].tile([128, 128], BF16, name="kt_ps",
                                           tag="h")
                nc.tensor.transpose(kt_ps[0:pn, :], kdT_bf[:, po:po + pn],
                                    id_b[:, :])
                kt_sb = cor2.tile([128, 128], BF16, name="kt_sb")
                nc.scalar.activation(kt_sb[0:pn, :], kt_ps[0:pn, :], AF.Copy)
                for (o, n) in CH:
                    nc.tensor.matmul(mt_ps[:, o:o + n], lhsT=kt_sb[0:pn, :],
                                     rhs=S[0:pn, o:o + n],
                                     start=(i == 0), stop=(i == 6))
            nc.scalar.activation(tiles["matchT"][:], mt_ps[:], AF.Copy)
            nc.vector.tensor_scalar_mul(tiles["matchT8"][:], mt_ps[:], 8.0)

            # positives: diag = qd . matched (own rows), summed
            posm = cor.tile([128, HW], F32, name="posm")
            nc.vector.tensor_mul(posm[:], qdT_bf[:], tiles["matchT"][:])
            pos_ps = pools["ps_m"].tile([1, HW], F32, name="pos_ps", tag="m")
            for (o, n) in CH:
                nc.tensor.matmul(pos_ps[:, o:o + n], lhsT=tiles["onesc"][:],
                                 rhs=posm[:, o:o + n], start=True, stop=True)
            nc.vector.reduce_sum(tiles["fin"][0:1, 0:1], pos_ps[:],
                                 axis=mybir.AxisListType.X)

        # ========== gathered q: dense logits with queue interleaved =====
        with tc.tile_pool(name="escr", bufs=3) as escr:
            # dense logits, column shard: all 6272 q rows x own 784 keys;
            # per-row exp sums via DVE (keeps the ACT chain pure Exp).
            # The 8 queue-negative groups are interleaved into the
            # ACT-bound phase so their PE/ACT work fills the slack.
            qe_sb = cst.tile([128, 512], BF16, name="qe_sb")

            def qe_group(grp):
                qe_ps = pools["ps_m"].tile([128, 64], F32, name="qe_ps",
                                           tag="m")
                for j in range(8):
                    qt = grp * 8 + j
                    nc.tensor.matmul(
                        qe_ps[:, j * 8:(j + 1) * 8],
                        lhsT=tiles["queueT8"][:, qt * 128:(qt + 1) * 128],
                        rhs=tiles["qgall"][:], start=(j == 0), stop=(j == 7))
                nc.scalar.activation(qe_sb[:, grp * 64:(grp + 1) * 64],
                                     qe_ps[:], AF.Exp, scale=ISC / 64.0)

            for t in range(RT):
                lg_ps = pools["ps_h"].tile([128, HW], F32, name="lg_ps",
                                           tag="h")
                for (o, n) in CH:
                    nc.tensor.matmul(
                        lg_ps[:, o:o + n],
                        lhsT=tiles["qall"][:, t * 128:(t + 1) * 128],
                        rhs=tiles["matchT8"][:, o:o + n],
                        start=True, stop=True)
                es = escr.tile([128, HW], BF16, name="es")
                nc.scalar.activation(es[:], lg_ps[:], AF.Exp,
                                     scale=ISC / 64.0)
                nc.vector.reduce_sum(tiles["zpart"][:, t:t + 1], es[:],
                                     axis=mybir.AxisListType.X)
                if t >= 13 and (t - 13) % 5 == 0 and (t - 13) // 5 < 8:
                    qe_group((t - 13) // 5)
            qs_ps = pools["ps_m"].tile([1, 512], F32, name="qs_ps", tag="m")
            nc.tensor.matmul(qs_ps[:], lhsT=tiles["onescb"][:], rhs=qe_sb[:],
                             start=True, stop=True)
            nc.vector.reduce_sum(tiles["fin"][0:1, 2:10],
                                 qs_ps[:].rearrange("p (t i) -> p i t", i=8),
                                 axis=mybir.AxisListType.X)

        nc.sync.dma_start(g("outz_d")[:], tiles["zpart"][:])
        nc.sync.dma_start(g("outs_d")[:], tiles["fin"][:])


def _prep_inputs(inputs):
    fq = np.asarray(inputs["feat_q"], np.float32).reshape(B, HW, C)
    fk = np.asarray(inputs["feat_k"], np.float32).reshape(B, HW, C)

    def xT8(x):  # (784, 1024) -> (128, 8*784) f8 with [c, ct*784+p]
        return np.ascontiguousarray(
            x.reshape(HW, CT, 128).transpose(2, 1, 0).reshape(128, CT * HW)
        ).astype(F8np)

    def w1tile(w):  # (1024, 2048) -> (16, 128, 1024) f8 x64
        return np.ascontiguousarray(
            (w * 64.0).reshape(CT, 128, DT, 128).transpose(2, 1, 0, 3)
            .reshape(DT, 128, C)).astype(F8np)

    def w2tile(w):  # (2048, 128) -> (128, 2048) f8 x64
        return np.ascontiguousarray(
            (w * 64.0).reshape(DT, 128, 128).transpose(1, 0, 2)
            .reshape(128, D)).astype(F8np)

    def wg1tile(w):  # (1024, 2048) -> (128, 8*2048) f8 x64
        return np.ascontiguousarray(
            (w * 64.0).reshape(CT, 128, D).transpose(1, 0, 2)
            .reshape(128, CT * D)).astype(F8np)

    shared = {
        "wd1": w1tile(inputs["Wd1"]), "wd1m": w1tile(inputs["mWd1"]),
        "wd2": w2tile(inputs["Wd2"]), "wd2m": w2tile(inputs["mWd2"]),
        "wg1": wg1tile(inputs["Wg1"]), "wg1m": wg1tile(inputs["mWg1"]),
        "wg2": w2tile(inputs["Wg2"]), "wg2m": w2tile(inputs["mWg2"]),
        "bd1": np.ascontiguousarray(
            np.asarray(inputs["bd1"], np.float32).reshape(DT, 128).T),
        "bd1m": np.ascontiguousarray(
            np.asarray(inputs["mbd1"], np.float32).reshape(DT, 128).T),
        "bd2": np.asarray(inputs["bd2"], np.float32).reshape(128, 1),
        "bd2m": np.asarray(inputs["mbd2"], np.float32).reshape(128, 1),
        "bg1r": (np.asarray(inputs["bg1"], np.float32) * 4096.0
                 ).reshape(1, D).astype(BFnp),
        "bg1mr": (np.asarray(inputs["mbg1"], np.float32) * 4096.0
                  ).reshape(1, D).astype(BFnp),
        "bg2": np.asarray(inputs["bg2"], np.float32).reshape(128, 1),
        "bg2m": np.asarray(inputs["mbg2"], np.float32).reshape(128, 1),
        "iota": np.arange(128, dtype=np.float32).reshape(128, 1),
        "onesc": np.ones((128, 1), np.float32),
        "onesr": np.ones((1, 128), np.float32),
        "ones8": np.ones((1, 8), np.float32).astype(BFnp),
    }
    queue = np.asarray(inputs["queue"], np.float32)
    in_maps = []
    for c in range(N_CORES):
        m = dict(shared)
        m["xq"] = xT8(fq[c])
        m["xk"] = xT8(fk[c])
        m["queueT"] = np.ascontiguousarray(
            (queue[c * QSH:(c + 1) * QSH] * 8.0).T).astype(F8np)
        in_maps.append(m)
    return in_maps


_NC = None


def _get_nc():
    global _NC
    if _NC is None:
        _NC = _build()
    return _NC


def _host_combine(outz, outs):
    """outz: [8][128, 49] z-partials; outs: [8][1, 16] scalars.

    outs slots: [0] sum(qd.matched) over own rows, [1] own-image lpos,
    [2:10] partial sum(exp(l_neg/tau)) per image over the core's queue
    shard.  Dense z row r=t*128+p lives at outz[:, p, t].
    """
    outz = np.asarray(outz, np.float64)   # [8, 128, 49]
    outs = np.asarray(outs, np.float64)   # [8, 16]
    z = outz.sum(axis=0)                  # [128, 49]
    zrows = z.T.reshape(-1)               # row r = t*128+p
    pos_total = outs[:, 0].sum()
    l_d = (np.log(zrows).sum() - ISC * pos_total) / NT
    zq = outs[:, 2:10].sum(axis=0)        # [8]
    lpos = outs[np.arange(8), 1]          # core c owns image c
    lse = np.log(zq + np.exp(ISC * lpos))
    l_g = np.mean(lse - ISC * lpos)
    return np.float32((1.0 - LAM) * l_g + LAM * l_d).reshape(())


def kernel(**inputs) -> np.ndarray:
    nc = _get_nc()
    in_maps = _prep_inputs(inputs)
    res = bass_utils.run_bass_kernel_spmd(nc, in_maps,
                                          core_ids=list(range(N_CORES)))
    outz = np.stack([res.results[c]["outz"] for c in range(N_CORES)])
    outs = np.stack([res.results[c]["outs"].reshape(16)
                     for c in range(N_CORES)])
    return _host_combine(outz, outs)
